# revision 5
# baseline (speedup 1.0000x reference)
"""Multi-head causal attention (B=4, S=2048, D=1024, H=16, RoPE) on 8 TRN2 cores.

Sharding: core = (batch b, head-group g of 8 heads).  Each core computes
qkv projection for its (b, g), RoPE, causal attention, and a partial
out-projection (contraction over its 512 head-dims).  Host sums the two
partials per batch.

Pipeline: qb-outer software pipeline; stage1(nb+1)/stage3(qb-1) units are
interleaved between stage2(qb) iterations as PE backfill (the j-loop is
exp/ACT-bound).  All matmuls run in bf16 (1 cyc/row at any free size).
RoPE rotate-half is a PE permutation matmul (cos/sin tables are invariant
under the 32-row swap).  The P@V matmul is emitted TRANSPOSED - output
[q-partitions, head-dims] - so all 128 output partitions are used (~2x
fewer PE cycles than [dims, q-free]) and the ones-column softmax
denominators land per-partition, where a reciprocal + tensor_scalar
normalize is cheap; a PE transpose then returns O^T to feature-major for
the out-projection.

Device layouts (per core):
  qk^T  [1024, S]  bf16 feature-major: tiles 0:4 = q (8 heads x 64), 4:8 = k
  v     [S, 520]   bf16 token-major, 65 cols/head: 64 dims + ones column
                   (accumulates softmax denominators during the P@V matmul)
  S^T   [t, q]     scores transposed; P tiles feed P@V as lhsT
  O^T   [512, S]   bf16 per-head outputs, feature-major, out-proj lhsT
  outp  [S, 1024]  fp16 partial (host sums the two cores of each batch)
"""

import math

import ml_dtypes
import numpy as np

import concourse.bacc as bacc
import concourse.mybir as mybir
from concourse import tile
from concourse.bass_utils import run_bass_kernel_spmd

AF = mybir.ActivationFunctionType
ALU = mybir.AluOpType
F32 = mybir.dt.float32
F32R = mybir.dt.float32r
BF16 = mybir.dt.bfloat16
BF16_NP = ml_dtypes.bfloat16

N_HEADS = 16
THETA = 10000.0
D = 1024
HD = 64
HL = 8          # heads per core
VW = HD + 1     # v columns per head (64 dims + ones)
NB = 512        # stage-1 token block
QB = 512        # query block
TT = 128        # key/value tile


def _host_constants(S):
    """RoPE tables, signed-swap permutation, causal mask (input-independent)."""
    half = HD // 2
    inv = 1.0 / (THETA ** (np.arange(half, dtype=np.float64) / half))
    t = np.arange(S, dtype=np.float64)
    ang = inv[:, None] * t[None, :]                      # [32, S]
    ropeC = np.tile(np.cos(ang), (4, 1)).astype(BF16_NP)   # [128, S]
    sinT = np.tile(np.sin(ang), (4, 1)).astype(BF16_NP)    # [128, S]

    # perm[k, m] = sig(m) iff k == swap(m): out[m] = sig(m) * in[swap(m)]
    perm = np.zeros((128, 128), dtype=BF16_NP)
    for m in range(128):
        blk, inner = (m // HD) * HD, m % HD
        partner = blk + (inner + half) % HD
        perm[partner, m] = -1.0 if inner < half else 1.0

    # maskT2[p, c] over two 128-col copies: upper-triangular keep (c >= p)
    p = np.arange(TT)[:, None]
    c = np.arange(TT)[None, :]
    m1 = (c >= p).astype(BF16_NP)
    maskT2 = np.concatenate([m1, m1], axis=1)            # [128, 256]
    return ropeC, sinT, perm, maskT2


def build_nc(S=2048):
    nc = bacc.Bacc("TRN2", target_bir_lowering=False, debug=False)

    xT = nc.dram_tensor("xT", [D, S], BF16, kind="ExternalInput").ap()
    wqkT = nc.dram_tensor("wqkT", [D, 2 * HL * HD], BF16, kind="ExternalInput").ap()
    wvT = nc.dram_tensor("wvT", [D, HL * HD], BF16, kind="ExternalInput").ap()
    woutT = nc.dram_tensor("woutT", [HL * HD, D], BF16, kind="ExternalInput").ap()
    outp = nc.dram_tensor("outp", [S, D], mybir.dt.float16, kind="ExternalOutput").ap()

    ropeC_np, sinT_np, perm_np, maskT2_np = _host_constants(S)
    ropeC_d = nc.inline_tensor(ropeC_np, "ropeC").ap()
    sinT_d = nc.inline_tensor(sinT_np, "sinT").ap()
    perm_d = nc.inline_tensor(perm_np, "perm").ap()
    maskT2_d = nc.inline_tensor(maskT2_np, "maskT2").ap()
    ident_d = nc.inline_tensor(np.eye(128, dtype=BF16_NP), "ident").ap()

    KD = D // 128        # 8 contraction tiles
    nNB = S // NB        # 4
    nQB = S // QB        # 4
    NTPB = QB // TT      # 4
    scale = 1.0 / math.sqrt(HD)
    PVLAG = 0

    with tile.TileContext(nc) as tc:
        with (
            tc.tile_pool(name="qk", bufs=1) as qk_pool,
            tc.tile_pool(name="vres", bufs=1) as v_pool,
            tc.tile_pool(name="osb", bufs=1) as o_pool,
            tc.tile_pool(name="wqk", bufs=1) as wqk_pool,
            tc.tile_pool(name="wv", bufs=1) as wv_pool,
            tc.tile_pool(name="wout", bufs=1) as wout_pool,
            tc.tile_pool(name="tabs", bufs=1) as tab_pool,
            tc.tile_pool(name="xs", bufs=2) as x_pool,
            tc.tile_pool(name="t1p", bufs=6) as t1_pool,
            tc.tile_pool(name="ptp", bufs=18) as pt_pool,
            tc.tile_pool(name="recp", bufs=3) as rec_pool,
            tc.tile_pool(name="t2p", bufs=4) as t2_pool,
            tc.tile_pool(name="otp", bufs=4) as ot_pool,
            tc.tile_pool(name="psA", bufs=2, space="PSUM") as psA,
            tc.tile_pool(name="psB", bufs=2, space="PSUM") as psB,
            tc.tile_pool(name="psO", bufs=1, space="PSUM") as psO,
        ):
            qk_sb = [qk_pool.tile([128, S], BF16, tag=f"qk{i}", name=f"qk{i}")
                     for i in range(8)]
            v_sb = [v_pool.tile([128, HL * VW], BF16, tag=f"v{i}", name=f"v{i}")
                    for i in range(S // TT)]
            o_sb = [o_pool.tile([128, S], BF16, tag=f"o{i}", name=f"o{i}")
                    for i in range(4)]
            wqkb = wqk_pool.tile([128, KD * 2 * HL * HD], BF16, tag="wqkb", name="wqkb")
            wvb = wv_pool.tile([128, KD * HL * HD], BF16, tag="wvb", name="wvb")
            wob = wout_pool.tile([128, 4 * D], BF16, tag="wob", name="wob")
            wqk_sb = [wqkb[:, k * 2 * HL * HD:(k + 1) * 2 * HL * HD] for k in range(KD)]
            wv_sb = [wvb[:, k * HL * HD:(k + 1) * HL * HD] for k in range(KD)]
            wout_sb = [wob[:, i * D:(i + 1) * D] for i in range(4)]
            ropeC_sb = tab_pool.tile([128, S], BF16, tag="ropeC", name="ropeC")
            sinT_sb = tab_pool.tile([128, S], BF16, tag="sinT", name="sinT")
            perm_sb = tab_pool.tile([128, 128], BF16, tag="perm", name="perm")
            maskT2_sb = tab_pool.tile([128, 2 * TT], BF16, tag="maskT2", name="maskT2")
            ident_sb = tab_pool.tile([128, 128], BF16, tag="ident", name="ident")

            xts = {}   # nb -> list of 8 [128, NB] f32r column slices

            xT_r = xT.rearrange("(k p) c -> p k c", k=KD)

            def load_x(nb, queue=nc.sync, pieces=1):
                tk = slice(nb * NB, (nb + 1) * NB)
                xb = x_pool.tile([128, KD * NB], BF16, name="xb")
                xb_r = xb[:].rearrange("p (k c) -> p k c", k=KD)
                kstep = KD // pieces
                for i in range(pieces):
                    ksl = slice(i * kstep, (i + 1) * kstep)
                    queue.dma_start(xb_r[:, ksl, :], xT_r[:, ksl, tk])
                xts[nb] = [xb[:, k * NB:(k + 1) * NB] for k in range(KD)]

            # ---------------- preamble: weights + first x block ----------
            # split across issue queues so the first matmul group starts fast
            wqkT_r = wqkT.rearrange("(k p) m -> p k m", k=KD)
            wqkb_r = wqkb[:].rearrange("p (k m) -> p k m", k=KD)
            wvT_r = wvT.rearrange("(k p) m -> p k m", k=KD)
            wvb_r = wvb[:].rearrange("p (k m) -> p k m", k=KD)
            woutT_r = woutT.rearrange("(i p) m -> p i m", i=4)
            wob_r = wob[:].rearrange("p (i m) -> p i m", i=4)
            # interleave x / wqk pieces so the first k-chain starts early
            xb0 = x_pool.tile([128, KD * NB], BF16, name="xb")
            xb0_r = xb0[:].rearrange("p (k c) -> p k c", k=KD)
            nc.scalar.dma_start(ropeC_sb[:], ropeC_d[:])
            nc.scalar.dma_start(sinT_sb[:], sinT_d[:])
            nc.scalar.dma_start(perm_sb[:], perm_d[:])
            for k0, k1 in ((0, 1), (1, 2), (2, 4), (4, 8)):
                ksl = slice(k0, k1)
                nc.sync.dma_start(xb0_r[:, ksl, :], xT_r[:, ksl, 0:NB])
                nc.sync.dma_start(wqkb_r[:, ksl, :], wqkT_r[:, ksl, :])
            xts[0] = [xb0[:, k * NB:(k + 1) * NB] for k in range(KD)]
            nc.scalar.dma_start(wvb_r[:], wvT_r[:])
            nc.scalar.dma_start(maskT2_sb[:], maskT2_d[:])
            nc.scalar.dma_start(ident_sb[:], ident_d[:])
            nc.scalar.dma_start(wob_r[:], woutT_r[:])

            # ---------------- stage 1: qkv projection + RoPE --------------
            def s1_rope(nb, mo, ps_ap, on_act=True):
                tok = slice(nb * NB, (nb + 1) * NB)
                dst = qk_sb[mo][:, tok]
                qs = psA.tile([128, NB], F32, tag="psA", name="qs")
                if on_act:
                    # PSUM read on ACT; bf16 SBUF DVE ops run in 2x mode
                    praw = t1_pool.tile([128, NB], BF16, tag="t1", name="praw")
                    nc.scalar.copy(praw[:], ps_ap)
                    nc.tensor.matmul(qs[:], lhsT=perm_sb[:], rhs=praw[:],
                                     start=True, stop=True)
                    nc.vector.tensor_tensor(dst, praw[:], ropeC_sb[:, tok],
                                            op=ALU.mult)
                    tm = t1_pool.tile([128, NB], BF16, tag="t1", name="tm")
                    nc.vector.tensor_tensor(tm[:], qs[:], sinT_sb[:, tok],
                                            op=ALU.mult)
                    nc.vector.tensor_tensor(dst, dst, tm[:], op=ALU.add)
                else:
                    # DVE-only variant (sin table is swap-invariant)
                    t1 = t1_pool.tile([128, NB], BF16, tag="t1", name="t1")
                    nc.vector.tensor_tensor(t1[:], ps_ap, sinT_sb[:, tok],
                                            op=ALU.mult)
                    nc.tensor.matmul(qs[:], lhsT=perm_sb[:], rhs=t1[:],
                                     start=True, stop=True)
                    nc.vector.tensor_tensor(dst, ps_ap, ropeC_sb[:, tok],
                                            op=ALU.mult)
                    nc.vector.tensor_tensor(dst, dst, qs[:], op=ALU.add)

            _qk_ps = {}

            def s1_qk_h(nb, mo, half):
                if half == 0:
                    _qk_ps[(nb, mo)] = psA.tile([128, NB], F32, tag="psA",
                                                name="ps")
                ps = _qk_ps[(nb, mo)]
                for k in range(half * 4, half * 4 + 4):
                    nc.tensor.matmul(
                        ps[:],
                        lhsT=wqk_sb[k][:, mo * 128:(mo + 1) * 128],
                        rhs=xts[nb][k][:],
                        start=(k == 0), stop=(k == KD - 1),
                    )
                if half == 1:
                    del _qk_ps[(nb, mo)]
                    if nb == 1:
                        on_act = True
                    elif nb == 3:
                        on_act = False
                    else:
                        on_act = (mo % 2 == 0)
                    s1_rope(nb, mo, ps[:], on_act=on_act)

            def s1_qk(nb, mo):
                s1_qk_h(nb, mo, 0)
                s1_qk_h(nb, mo, 1)

            def s1_v_mm(nb, mt, pv_ap, ks):
                xsl = slice(mt * 128, (mt + 1) * 128)
                for k in ks:
                    nc.tensor.matmul(
                        pv_ap,
                        lhsT=xts[nb][k][:, xsl],
                        rhs=wv_sb[k],
                        start=(k == 0), stop=(k == KD - 1),
                    )

            def s1_v_fin(nb, mt, pv, on_act=False):
                vt = v_sb[nb * (NB // TT) + mt]
                vre = vt[:].rearrange("p (h c) -> p h c", h=HL)
                if on_act:
                    nc.scalar.copy(
                        vre[:, :, 0:HD], pv[:].rearrange("p (h c) -> p h c", h=HL))
                else:
                    nc.vector.tensor_copy(
                        vre[:, :, 0:HD], pv[:].rearrange("p (h c) -> p h c", h=HL))
                nc.vector.memset(vre[:, :, HD], 1.0)

            def s1_v(nb, mt, on_act=False):
                pv = psA.tile([128, HL * HD], F32, tag="psA", name="pv")
                s1_v_mm(nb, mt, pv[:], range(KD))
                s1_v_fin(nb, mt, pv, on_act=on_act)

            # ---------------- stage 2: causal attention ------------------
            def s2_scores(qb, hp, j, pts):
                qt = qk_sb[hp]
                kt = qk_sb[4 + hp]
                oi = j - NTPB * qb
                c0 = max(oi, 0) * TT
                st = psB.tile([128, 2 * QB], F32, tag="psB", name="st")
                for hi in (0, 1):
                    base = hi * HD
                    nc.tensor.matmul(
                        st[:, hi * QB + c0:(hi + 1) * QB],
                        lhsT=kt[base:base + HD, j * TT:(j + 1) * TT],
                        rhs=qt[base:base + HD, qb * QB + c0:(qb + 1) * QB],
                        start=True, stop=True,
                    )
                pt = pt_pool.tile([128, 2 * QB], BF16, name="pt")
                st2 = st[:].rearrange("p (h c) -> p h c", h=2)
                pt2 = pt[:].rearrange("p (h c) -> p h c", h=2)
                nc.scalar.activation(pt2[:, :, c0:QB], st2[:, :, c0:QB],
                                     AF.Exp, scale=scale)
                if oi >= 0:
                    for hi in (0, 1):
                        csl = slice(hi * QB + c0, hi * QB + c0 + TT)
                        nc.vector.tensor_tensor(
                            pt[:, csl], pt[:, csl],
                            maskT2_sb[:, hi * TT:(hi + 1) * TT],
                            op=ALU.mult)
                pts[j] = pt

            def s2_pv_group(qb, hp, sub, pts, oaug8, rec, o_qm):
                # full accumulation chain for q-subblock `sub` (both heads),
                # followed immediately by its reciprocal + normalize
                h0, h1 = 2 * hp, 2 * hp + 1
                jlast = NTPB * qb + sub
                for hi, hh in ((0, h0), (1, h1)):
                    for j in range(jlast + 1):
                        nc.tensor.matmul(
                            oaug8[hi][:, VW * sub:VW * sub + VW],
                            lhsT=pts[j][:, hi * QB + sub * TT:hi * QB + (sub + 1) * TT],
                            rhs=v_sb[j][:, VW * hh:VW * hh + VW],
                            start=(j == 0), stop=(j == jlast),
                        )
                for hi in (0, 1):
                    g = hi * NTPB + sub
                    nc.vector.reciprocal(
                        rec[:, g:g + 1],
                        oaug8[hi][:, VW * sub + HD:VW * sub + VW])
                    nc.vector.tensor_scalar_mul(
                        o_qm[:, sub * TT + hi * HD:sub * TT + hi * HD + HD],
                        oaug8[hi][:, VW * sub:VW * sub + HD],
                        rec[:, g:g + 1])

            def s2_tail(qb, hp, o_qm):
                # PE transpose back to dim-major + one copy out
                tp = psA.tile([128, QB], BF16, tag="psA", name="tp")
                for sub in range(NTPB):
                    nc.tensor.transpose(
                        tp[:, sub * TT:(sub + 1) * TT],
                        o_qm[:, sub * TT:(sub + 1) * TT], ident_sb[:])
                qsl = slice(qb * QB, (qb + 1) * QB)
                if hp % 2 == 0:
                    nc.scalar.copy(o_sb[hp][:, qsl], tp[:])
                else:
                    nc.vector.tensor_copy(o_sb[hp][:, qsl], tp[:])

            def run_qb(qb, fillers, slots=None, late=()):
                """Emit one qb phase: lag-2 PV issue + filler interleave."""
                jmax = NTPB * (qb + 1)
                total = 4 * jmax
                if slots is None:
                    slots = [int(round((i + 1) * total / (len(fillers) + 1)))
                             for i in range(len(fillers))]
                fi = 0
                slot = 0
                for hp in range(4):
                    oaug8 = [psO.tile([128, NTPB * VW], F32, tag="psO",
                                      name="oaug") for _ in range(2)]
                    rec = rec_pool.tile([128, 2 * NTPB], F32, tag="rec8",
                                        name="rec")
                    o_qm = t2_pool.tile([128, QB], BF16, tag="t2", name="oqm")
                    for j in range(jmax):
                        s2_scores(qb, hp, j, pend_pts)
                        oi = j - NTPB * qb - PVLAG
                        if oi >= 0:
                            s2_pv_group(qb, hp, oi, pend_pts, oaug8, rec, o_qm)
                        slot += 1
                        while fi < len(fillers) and slots[fi] <= slot:
                            fillers[fi]()
                            fi += 1
                    for oi in range(NTPB - PVLAG, NTPB):
                        s2_pv_group(qb, hp, oi, pend_pts, oaug8, rec, o_qm)
                    pend_pts.clear()
                    s2_tail(qb, hp, o_qm)
                    if hp == 3:
                        for f in late:
                            f()
                while fi < len(fillers):
                    fillers[fi]()
                    fi += 1

            pend_pts = {}

            # ---------------- stage 3: out projection ---------------------
            _s3_ot = {}

            def s3_ib(qb, mt, ib, alt=False):
                    tsl = slice(qb * QB + mt * 128, qb * QB + (mt + 1) * 128)
                    if ib == 0:
                        _s3_ot[(qb, mt)] = ot_pool.tile(
                            [128, D], mybir.dt.float16, name="ot")
                    ot = _s3_ot[(qb, mt)]
                    po = psB.tile([128, 512], F32, tag="psB", name="po")
                    for hp in range(4):
                        nc.tensor.matmul(
                            po[:],
                            lhsT=o_sb[hp][:, tsl],
                            rhs=wout_sb[hp][:, ib * 512:(ib + 1) * 512],
                            start=(hp == 0), stop=(hp == 3),
                        )
                    osl = slice(ib * 512, (ib + 1) * 512)
                    if alt and ib == 0:
                        nc.scalar.copy(ot[:, osl], po[:])
                    else:
                        nc.vector.tensor_copy(ot[:, osl], po[:])
                    if ib == 1:
                        del _s3_ot[(qb, mt)]
                        nc.sync.dma_start(outp[tsl, :], ot[:])

            def s3_mt(qb, mt, alt=False):
                    s3_ib(qb, mt, 0, alt)
                    s3_ib(qb, mt, 1, alt)

            # ---------------- pipelined emission --------------------------
            # nb=0: 8 concurrent groups (6 qk-mo on psA x2 + psB halves x4,
            # 2 v-tiles on psO), k-interleaved to match the x/wqk DMA pieces
            ps0 = psA.tile([128, NB], F32, tag="psA", name="ps")
            ps1 = psA.tile([128, NB], F32, tag="psA", name="ps")
            stb0 = psB.tile([128, 2 * QB], F32, tag="psB", name="st")
            stb1 = psB.tile([128, 2 * QB], F32, tag="psB", name="st")
            pv0 = psO.tile([128, HL * HD], F32, tag="psO", name="oaug")
            g_aps = [ps0[:], ps1[:], stb0[:, 0:512], stb0[:, 512:1024],
                     stb1[:, 0:512], stb1[:, 512:1024]]
            for ks in ((0, 1), (2, 3), (4, 5), (6, 7)):
                for g in range(6):
                    for k in ks:
                        nc.tensor.matmul(
                            g_aps[g],
                            lhsT=wqk_sb[k][:, g * 128:(g + 1) * 128],
                            rhs=xts[0][k],
                            start=(k == 0), stop=(k == KD - 1),
                        )
                s1_v_mm(0, 0, pv0[:], ks)
            # ropes interleaved with the remaining PE work (mo6/7, v2/3)
            s1_rope(0, 0, g_aps[0], on_act=True)
            s1_rope(0, 1, g_aps[1], on_act=True)
            s1_v_fin(0, 0, pv0, on_act=True)
            s1_rope(0, 2, g_aps[2], on_act=True)
            s1_qk(0, 6)
            s1_rope(0, 3, g_aps[3], on_act=True)
            s1_qk(0, 7)
            s1_rope(0, 4, g_aps[4], on_act=True)
            s1_v(0, 1, on_act=True)
            s1_rope(0, 5, g_aps[5], on_act=True)
            s1_v(0, 2, on_act=True)
            s1_v(0, 3, on_act=True)
            load_x(1, pieces=2)
            # qb=0: overlap with full s1(1)
            f0 = []
            for m in range(8):
                f0 += [lambda m=m: s1_qk_h(1, m, 0), lambda m=m: s1_qk_h(1, m, 1)]
                if m % 2 == 1:
                    f0.append(lambda m=m: s1_v(1, m // 2))
            run_qb(0, f0)
            load_x(2, pieces=2)
            # qb=1: overlap with full s1(2) + s3(0)
            f1 = []
            for m in range(8):
                f1 += [lambda m=m: s1_qk_h(2, m, 0), lambda m=m: s1_qk_h(2, m, 1)]
                if m % 2 == 1:
                    f1.append(lambda m=m: s1_v(2, m // 2))
                    f1.append(lambda m=m: s3_ib(0, m // 2, 0))
                    f1.append(lambda m=m: s3_ib(0, m // 2, 1))
            run_qb(1, f1)
            load_x(3, pieces=2)
            # qb=2: overlap with s1(3) q-projection + s3(1)
            f2 = []
            for m in range(4):
                f2 += [lambda m=m: s1_qk_h(3, m, 0), lambda m=m: s1_qk_h(3, m, 1)]
                f2 += [lambda m=m: s3_ib(1, m, 0), lambda m=m: s3_ib(1, m, 1)]
            run_qb(2, f2)
            # qb=3: backfill with deferred s1(3) k-projection + v + s3(2)
            f3 = [lambda: s1_qk_h(3, 4, 0), lambda: s1_qk_h(3, 4, 1)]
            f3 += [lambda m=m: s1_v(3, m) for m in range(4)]
            for m in (1, 2, 3):
                f3 += [lambda m=m: s1_qk_h(3, 4 + m, 0),
                       lambda m=m: s1_qk_h(3, 4 + m, 1)]
            f3 += [lambda m=m: s3_ib(2, m // 2, m % 2) for m in range(4)]
            # front-load k(hp0) + all v tiles so hp0's diagonal PVs are fed;
            # hold two s3(2) units back to cover the last tail chain
            run_qb(3, f3, slots=[1, 2, 4, 5, 7, 8, 12, 16, 20, 26, 32, 38,
                                 44, 50, 54, 58],
                   late=[lambda m=m: s3_mt(2, m) for m in (2, 3)])
            for mt in range(4):
                s3_mt(3, mt, alt=True)

    nc.compile()
    return nc


# ---------------------------------------------------------------------------
# host side
# ---------------------------------------------------------------------------

_cache = {}


def _get_nc(S):
    if S not in _cache:
        _cache[S] = build_nc(S)
    return _cache[S]


def _shard_weights(w_qkv, w_out, g):
    """Per-head-group weight shards in device layouts."""
    w_qkv = np.asarray(w_qkv, dtype=np.float32)
    w_out = np.asarray(w_out, dtype=np.float32)
    r = slice(g * 512, (g + 1) * 512)
    wq = w_qkv[0:D][r]            # [512, 1024]
    wk = w_qkv[D:2 * D][r]
    wv = w_qkv[2 * D:3 * D][r]
    wqkT = np.ascontiguousarray(np.concatenate([wq, wk], axis=0).T).astype(BF16_NP)
    wvT = np.ascontiguousarray(wv.T).astype(BF16_NP)                  # [1024, 512]
    woutT = np.ascontiguousarray(w_out.T[r]).astype(BF16_NP)          # [512, 1024]
    return wqkT, wvT, woutT


def kernel(x, w_qkv, w_out):
    x = np.asarray(x, dtype=np.float32)
    B, S, _D = x.shape
    assert _D == D
    nc = _get_nc(S)

    shards = [_shard_weights(w_qkv, w_out, g) for g in range(2)]
    in_maps = []
    for core in range(8):
        b, g = core // 2, core % 2
        wqkT, wvT, woutT = shards[g]
        in_maps.append({
            "xT": np.ascontiguousarray(x[b].T).astype(BF16_NP),
            "wqkT": wqkT,
            "wvT": wvT,
            "woutT": woutT,
        })
    res = run_bass_kernel_spmd(nc, in_maps, list(range(8)))
    out = np.empty((B, S, D), dtype=np.float32)
    for b in range(B):
        out[b] = (res.results[2 * b]["outp"].astype(np.float32)
                  + res.results[2 * b + 1]["outp"].astype(np.float32))
    return out


# revision 8
# speedup vs baseline: 1.0503x; 1.0503x over previous
"""Multi-head causal attention (B=4, S=2048, D=1024, H=16, RoPE) on 8 TRN2 cores.

Sharding: core = (batch b, head-group g of 8 heads).  Each core computes
qkv projection for its (b, g), RoPE, causal attention, and a partial
out-projection (contraction over its 512 head-dims).  Host sums the two
partials per batch.

Pipeline: qb-outer software pipeline; stage1(nb+1)/stage3(qb-1) units are
interleaved between stage2(qb) iterations as PE backfill (the j-loop is
exp/ACT-bound).  All matmuls run in bf16 (1 cyc/row at any free size).
RoPE rotate-half is a PE permutation matmul (cos/sin tables are invariant
under the 32-row swap).  The P@V matmul is emitted TRANSPOSED - output
[q-partitions, head-dims] - so all 128 output partitions are used (~2x
fewer PE cycles than [dims, q-free]) and the ones-column softmax
denominators land per-partition, where a reciprocal + tensor_scalar
normalize is cheap; a PE transpose then returns O^T to feature-major for
the out-projection.

Device layouts (per core):
  qk^T  [1024, S]  bf16 feature-major: tiles 0:4 = q (8 heads x 64), 4:8 = k
  v     [S, 520]   bf16 token-major, 65 cols/head: 64 dims + ones column
                   (accumulates softmax denominators during the P@V matmul)
  S^T   [t, q]     scores transposed; P tiles feed P@V as lhsT
  O^T   [512, S]   bf16 per-head outputs, feature-major, out-proj lhsT
  outp  [S, 1024]  fp16 partial (host sums the two cores of each batch)
"""

import math

import ml_dtypes
import numpy as np

import concourse.bacc as bacc
import concourse.mybir as mybir
from concourse import tile
from concourse.bass_utils import run_bass_kernel_spmd

AF = mybir.ActivationFunctionType
ALU = mybir.AluOpType
F32 = mybir.dt.float32
F32R = mybir.dt.float32r
BF16 = mybir.dt.bfloat16
BF16_NP = ml_dtypes.bfloat16

N_HEADS = 16
THETA = 10000.0
D = 1024
HD = 64
HL = 8          # heads per core
VW = HD + 1     # v columns per head (64 dims + ones)
NB = 512        # stage-1 token block
QB = 512        # query block
TT = 128        # key/value tile


def _host_constants(S):
    """RoPE tables, signed-swap permutation, causal mask (input-independent)."""
    half = HD // 2
    inv = 1.0 / (THETA ** (np.arange(half, dtype=np.float64) / half))
    t = np.arange(S, dtype=np.float64)
    ang = inv[:, None] * t[None, :]                      # [32, S]
    ropeC = np.tile(np.cos(ang), (4, 1)).astype(BF16_NP)   # [128, S]
    sinT = np.tile(np.sin(ang), (4, 1)).astype(BF16_NP)    # [128, S]

    # perm[k, m] = sig(m) iff k == swap(m): out[m] = sig(m) * in[swap(m)]
    perm = np.zeros((128, 128), dtype=BF16_NP)
    for m in range(128):
        blk, inner = (m // HD) * HD, m % HD
        partner = blk + (inner + half) % HD
        perm[partner, m] = -1.0 if inner < half else 1.0

    # maskT2[p, c] over two 128-col copies: upper-triangular keep (c >= p)
    p = np.arange(TT)[:, None]
    c = np.arange(TT)[None, :]
    m1 = (c >= p).astype(BF16_NP)
    maskT2 = np.concatenate([m1, m1], axis=1)            # [128, 256]
    return ropeC, sinT, perm, maskT2


def build_nc(S=2048):
    nc = bacc.Bacc("TRN2", target_bir_lowering=False, debug=False)

    xT = nc.dram_tensor("xT", [D, S], BF16, kind="ExternalInput").ap()
    wqkT = nc.dram_tensor("wqkT", [D, 2 * HL * HD], BF16, kind="ExternalInput").ap()
    wvT = nc.dram_tensor("wvT", [D, HL * HD], BF16, kind="ExternalInput").ap()
    woutT = nc.dram_tensor("woutT", [HL * HD, D], BF16, kind="ExternalInput").ap()
    outp = nc.dram_tensor("outp", [S, D], mybir.dt.float16, kind="ExternalOutput").ap()

    ropeC_np, sinT_np, perm_np, maskT2_np = _host_constants(S)
    ropeC_d = nc.inline_tensor(ropeC_np, "ropeC").ap()
    sinT_d = nc.inline_tensor(sinT_np, "sinT").ap()
    perm_d = nc.inline_tensor(perm_np, "perm").ap()
    maskT2_d = nc.inline_tensor(maskT2_np, "maskT2").ap()
    ident_d = nc.inline_tensor(np.eye(128, dtype=BF16_NP), "ident").ap()

    KD = D // 128        # 8 contraction tiles
    nNB = S // NB        # 4
    nQB = S // QB        # 4
    NTPB = QB // TT      # 4
    scale = 1.0 / math.sqrt(HD)
    PVLAG = 0

    with tile.TileContext(nc) as tc:
        with (
            tc.tile_pool(name="qk", bufs=1) as qk_pool,
            tc.tile_pool(name="vres", bufs=1) as v_pool,
            tc.tile_pool(name="osb", bufs=1) as o_pool,
            tc.tile_pool(name="wqk", bufs=1) as wqk_pool,
            tc.tile_pool(name="wv", bufs=1) as wv_pool,
            tc.tile_pool(name="wout", bufs=1) as wout_pool,
            tc.tile_pool(name="tabs", bufs=1) as tab_pool,
            tc.tile_pool(name="xs", bufs=2) as x_pool,
            tc.tile_pool(name="t1p", bufs=6) as t1_pool,
            tc.tile_pool(name="ptp", bufs=26) as pt_pool,
            tc.tile_pool(name="recp", bufs=3) as rec_pool,
            tc.tile_pool(name="t2p", bufs=4) as t2_pool,
            tc.tile_pool(name="otp", bufs=4) as ot_pool,
            tc.tile_pool(name="psA", bufs=2, space="PSUM") as psA,
            tc.tile_pool(name="psB", bufs=3, space="PSUM") as psB,
        ):
            qk_sb = [qk_pool.tile([128, S], BF16, tag=f"qk{i}", name=f"qk{i}")
                     for i in range(8)]
            v_sb = [v_pool.tile([128, HL * VW], BF16, tag=f"v{i}", name=f"v{i}")
                    for i in range(S // TT)]
            o_sb = [o_pool.tile([128, S], BF16, tag=f"o{i}", name=f"o{i}")
                    for i in range(4)]
            wqkb = wqk_pool.tile([128, KD * 2 * HL * HD], BF16, tag="wqkb", name="wqkb")
            wvb = wv_pool.tile([128, KD * HL * HD], BF16, tag="wvb", name="wvb")
            wob = wout_pool.tile([128, 4 * D], BF16, tag="wob", name="wob")
            wqk_sb = [wqkb[:, k * 2 * HL * HD:(k + 1) * 2 * HL * HD] for k in range(KD)]
            wv_sb = [wvb[:, k * HL * HD:(k + 1) * HL * HD] for k in range(KD)]
            wout_sb = [wob[:, i * D:(i + 1) * D] for i in range(4)]
            ropeC_sb = tab_pool.tile([128, S], BF16, tag="ropeC", name="ropeC")
            sinT_sb = tab_pool.tile([128, S], BF16, tag="sinT", name="sinT")
            perm_sb = tab_pool.tile([128, 128], BF16, tag="perm", name="perm")
            maskT2_sb = tab_pool.tile([128, 2 * TT], BF16, tag="maskT2", name="maskT2")
            ident_sb = tab_pool.tile([128, 128], BF16, tag="ident", name="ident")

            xts = {}   # nb -> list of 8 [128, NB] f32r column slices

            xT_r = xT.rearrange("(k p) c -> p k c", k=KD)

            def load_x(nb, queue=nc.sync, pieces=1):
                tk = slice(nb * NB, (nb + 1) * NB)
                xb = x_pool.tile([128, KD * NB], BF16, name="xb")
                xb_r = xb[:].rearrange("p (k c) -> p k c", k=KD)
                kstep = KD // pieces
                for i in range(pieces):
                    ksl = slice(i * kstep, (i + 1) * kstep)
                    queue.dma_start(xb_r[:, ksl, :], xT_r[:, ksl, tk])
                xts[nb] = [xb[:, k * NB:(k + 1) * NB] for k in range(KD)]

            # ---------------- preamble: weights + first x block ----------
            # split across issue queues so the first matmul group starts fast
            wqkT_r = wqkT.rearrange("(k p) m -> p k m", k=KD)
            wqkb_r = wqkb[:].rearrange("p (k m) -> p k m", k=KD)
            wvT_r = wvT.rearrange("(k p) m -> p k m", k=KD)
            wvb_r = wvb[:].rearrange("p (k m) -> p k m", k=KD)
            woutT_r = woutT.rearrange("(i p) m -> p i m", i=4)
            wob_r = wob[:].rearrange("p (i m) -> p i m", i=4)
            # interleave x / wqk pieces so the first k-chain starts early
            xb0 = x_pool.tile([128, KD * NB], BF16, name="xb")
            xb0_r = xb0[:].rearrange("p (k c) -> p k c", k=KD)
            nc.scalar.dma_start(ropeC_sb[:], ropeC_d[:])
            nc.scalar.dma_start(sinT_sb[:], sinT_d[:])
            nc.scalar.dma_start(perm_sb[:], perm_d[:])
            for k0, k1 in ((0, 1), (1, 2), (2, 4), (4, 8)):
                ksl = slice(k0, k1)
                nc.sync.dma_start(xb0_r[:, ksl, :], xT_r[:, ksl, 0:NB])
                nc.sync.dma_start(wqkb_r[:, ksl, :], wqkT_r[:, ksl, :])
            xts[0] = [xb0[:, k * NB:(k + 1) * NB] for k in range(KD)]
            nc.scalar.dma_start(wvb_r[:], wvT_r[:])
            nc.scalar.dma_start(maskT2_sb[:], maskT2_d[:])
            nc.scalar.dma_start(ident_sb[:], ident_d[:])
            nc.scalar.dma_start(wob_r[:], woutT_r[:])

            # ---------------- stage 1: qkv projection + RoPE --------------
            def s1_rope(nb, mo, ps_ap, on_act=True):
                tok = slice(nb * NB, (nb + 1) * NB)
                dst = qk_sb[mo][:, tok]
                qs = psA.tile([128, NB], F32, tag="psA", name="qs")
                if on_act:
                    # PSUM read on ACT; bf16 SBUF DVE ops run in 2x mode
                    praw = t1_pool.tile([128, NB], BF16, tag="t1", name="praw")
                    nc.scalar.copy(praw[:], ps_ap)
                    nc.tensor.matmul(qs[:], lhsT=perm_sb[:], rhs=praw[:],
                                     start=True, stop=True)
                    nc.vector.tensor_tensor(dst, praw[:], ropeC_sb[:, tok],
                                            op=ALU.mult)
                    tm = t1_pool.tile([128, NB], BF16, tag="t1", name="tm")
                    nc.vector.tensor_tensor(tm[:], qs[:], sinT_sb[:, tok],
                                            op=ALU.mult)
                    nc.vector.tensor_tensor(dst, dst, tm[:], op=ALU.add)
                else:
                    # DVE-only variant (sin table is swap-invariant)
                    t1 = t1_pool.tile([128, NB], BF16, tag="t1", name="t1")
                    nc.vector.tensor_tensor(t1[:], ps_ap, sinT_sb[:, tok],
                                            op=ALU.mult)
                    nc.tensor.matmul(qs[:], lhsT=perm_sb[:], rhs=t1[:],
                                     start=True, stop=True)
                    nc.vector.tensor_tensor(dst, ps_ap, ropeC_sb[:, tok],
                                            op=ALU.mult)
                    nc.vector.tensor_tensor(dst, dst, qs[:], op=ALU.add)

            _qk_ps = {}

            def s1_qk_h(nb, mo, half):
                if half == 0:
                    _qk_ps[(nb, mo)] = psA.tile([128, NB], F32, tag="psA",
                                                name="ps")
                ps = _qk_ps[(nb, mo)]
                for k in range(half * 4, half * 4 + 4):
                    nc.tensor.matmul(
                        ps[:],
                        lhsT=wqk_sb[k][:, mo * 128:(mo + 1) * 128],
                        rhs=xts[nb][k][:],
                        start=(k == 0), stop=(k == KD - 1),
                    )
                if half == 1:
                    del _qk_ps[(nb, mo)]
                    if nb == 1:
                        on_act = True
                    elif nb == 3:
                        on_act = False
                    else:
                        on_act = (mo % 2 == 0)
                    s1_rope(nb, mo, ps[:], on_act=on_act)

            def s1_qk(nb, mo):
                s1_qk_h(nb, mo, 0)
                s1_qk_h(nb, mo, 1)

            def s1_v_mm(nb, mt, pv_ap, ks):
                xsl = slice(mt * 128, (mt + 1) * 128)
                for k in ks:
                    nc.tensor.matmul(
                        pv_ap,
                        lhsT=xts[nb][k][:, xsl],
                        rhs=wv_sb[k],
                        start=(k == 0), stop=(k == KD - 1),
                    )

            def s1_v_fin(nb, mt, pv, on_act=False):
                vt = v_sb[nb * (NB // TT) + mt]
                vre = vt[:].rearrange("p (h c) -> p h c", h=HL)
                if on_act:
                    nc.scalar.copy(
                        vre[:, :, 0:HD], pv[:].rearrange("p (h c) -> p h c", h=HL))
                else:
                    nc.vector.tensor_copy(
                        vre[:, :, 0:HD], pv[:].rearrange("p (h c) -> p h c", h=HL))
                nc.vector.memset(vre[:, :, HD], 1.0)

            def s1_v(nb, mt, on_act=False):
                pv = psA.tile([128, HL * HD], F32, tag="psA", name="pv")
                s1_v_mm(nb, mt, pv[:], range(KD))
                s1_v_fin(nb, mt, pv, on_act=on_act)

            # ---------------- stage 2: causal attention ------------------
            def s2_scores(qb, hp, j, pts):
                qt = qk_sb[hp]
                kt = qk_sb[4 + hp]
                oi = j - NTPB * qb
                c0 = max(oi, 0) * TT
                st = psB.tile([128, 2 * QB], F32, tag="psB", name="st")
                for hi in (0, 1):
                    base = hi * HD
                    nc.tensor.matmul(
                        st[:, hi * QB + c0:(hi + 1) * QB],
                        lhsT=kt[base:base + HD, j * TT:(j + 1) * TT],
                        rhs=qt[base:base + HD, qb * QB + c0:(qb + 1) * QB],
                        start=True, stop=True,
                    )
                pt = pt_pool.tile([128, 2 * QB], BF16, name="pt")
                st2 = st[:].rearrange("p (h c) -> p h c", h=2)
                pt2 = pt[:].rearrange("p (h c) -> p h c", h=2)
                nc.scalar.activation(pt2[:, :, c0:QB], st2[:, :, c0:QB],
                                     AF.Exp, scale=scale)
                if oi >= 0:
                    eng = nc.gpsimd if qb >= 2 else nc.vector
                    for hi in (0, 1):
                        csl = slice(hi * QB + c0, hi * QB + c0 + TT)
                        eng.tensor_tensor(
                            pt[:, csl], pt[:, csl],
                            maskT2_sb[:, hi * TT:(hi + 1) * TT],
                            op=ALU.mult)
                pts[(hp, j)] = pt

            def s2_pv_group(qb, hp, sub, pts, oaug8, rec, o_qm):
                # full accumulation chain for q-subblock `sub` (both heads),
                # followed immediately by its reciprocal + normalize
                h0, h1 = 2 * hp, 2 * hp + 1
                jlast = NTPB * qb + sub
                for hi, hh in ((0, h0), (1, h1)):
                    g = hi * NTPB + sub
                    for j in range(jlast + 1):
                        nc.tensor.matmul(
                            oaug8[:, TT * g:TT * g + VW],
                            lhsT=pts[(hp, j)][:, hi * QB + sub * TT:hi * QB + (sub + 1) * TT],
                            rhs=v_sb[j][:, VW * hh:VW * hh + VW],
                            start=(j == 0), stop=(j == jlast),
                        )
                for hi in (0, 1):
                    g = hi * NTPB + sub
                    nc.vector.reciprocal(
                        rec[:, g:g + 1],
                        oaug8[:, TT * g + HD:TT * g + VW])
                    nc.vector.tensor_scalar_mul(
                        o_qm[:, sub * TT + hi * HD:sub * TT + hi * HD + HD],
                        oaug8[:, TT * g:TT * g + HD],
                        rec[:, g:g + 1])

            def s2_tail(qb, hp, o_qm):
                # PE transpose back to dim-major + one copy out
                tp = psA.tile([128, QB], BF16, tag="psA", name="tp")
                for sub in range(NTPB):
                    nc.tensor.transpose(
                        tp[:, sub * TT:(sub + 1) * TT],
                        o_qm[:, sub * TT:(sub + 1) * TT], ident_sb[:])
                qsl = slice(qb * QB, (qb + 1) * QB)
                if qb < 2:
                    nc.scalar.copy(o_sb[hp][:, qsl], tp[:])
                else:
                    nc.vector.tensor_copy(o_sb[hp][:, qsl], tp[:])

            def run_qb(qb, fillers, slots=None, late=()):
                """Emit one qb phase: lag-2 PV issue + filler interleave."""
                jmax = NTPB * (qb + 1)
                total = 4 * jmax
                if slots is None:
                    slots = [int(round((i + 1) * total / (len(fillers) + 1)))
                             for i in range(len(fillers))]
                fi = 0
                slot = 0
                prewarm = (3, 3, 3, 3)[qb]
                for hp in range(4):
                    oaug8 = psB.tile([128, 2 * QB], F32, tag="psB",
                                     name="oaug")
                    rec = rec_pool.tile([128, 2 * NTPB], F32, tag="rec8",
                                        name="rec")
                    o_qm = t2_pool.tile([128, QB], BF16, tag="t2", name="oqm")
                    start_j = prewarm if hp > 0 else 0
                    # groups whose diagonal falls before start_j (their
                    # scores were prewarmed) must still be emitted
                    for oi in range(0, start_j - NTPB * qb):
                        s2_pv_group(qb, hp, oi, pend_pts, oaug8, rec, o_qm)
                    for j in range(start_j, jmax):
                        s2_scores(qb, hp, j, pend_pts)
                        oi = j - NTPB * qb - PVLAG
                        if oi >= 0:
                            s2_pv_group(qb, hp, oi, pend_pts, oaug8, rec, o_qm)
                        slot += 1
                        while fi < len(fillers) and slots[fi] <= slot:
                            fillers[fi]()
                            fi += 1
                    for oi in range(NTPB - PVLAG, NTPB):
                        s2_pv_group(qb, hp, oi, pend_pts, oaug8, rec, o_qm)
                    # prewarm the next hp's first scores/exps: ACT chews them
                    # while this hp's PV chains and tail run on PE/DVE
                    if hp < 3:
                        for j2 in range(prewarm):
                            s2_scores(qb, hp + 1, j2, pend_pts)
                    for k in [k for k in pend_pts if k[0] == hp]:
                        del pend_pts[k]
                    s2_tail(qb, hp, o_qm)
                    if hp == 3:
                        for f in late:
                            f()
                while fi < len(fillers):
                    fillers[fi]()
                    fi += 1

            pend_pts = {}

            # ---------------- stage 3: out projection ---------------------
            _s3_ot = {}

            def s3_ib(qb, mt, ib, alt=False):
                    tsl = slice(qb * QB + mt * 128, qb * QB + (mt + 1) * 128)
                    if ib == 0:
                        _s3_ot[(qb, mt)] = ot_pool.tile(
                            [128, D], mybir.dt.float16, name="ot")
                    ot = _s3_ot[(qb, mt)]
                    po = psB.tile([128, 512], F32, tag="psB", name="po")
                    for hp in range(4):
                        nc.tensor.matmul(
                            po[:],
                            lhsT=o_sb[hp][:, tsl],
                            rhs=wout_sb[hp][:, ib * 512:(ib + 1) * 512],
                            start=(hp == 0), stop=(hp == 3),
                        )
                    osl = slice(ib * 512, (ib + 1) * 512)
                    if alt and ib == 0:
                        nc.scalar.copy(ot[:, osl], po[:])
                    else:
                        nc.vector.tensor_copy(ot[:, osl], po[:])
                    if ib == 1:
                        del _s3_ot[(qb, mt)]
                        nc.sync.dma_start(outp[tsl, :], ot[:])

            def s3_mt(qb, mt, alt=False):
                    s3_ib(qb, mt, 0, alt)
                    s3_ib(qb, mt, 1, alt)

            # ---------------- pipelined emission --------------------------
            # nb=0: 8 concurrent groups (6 qk-mo on psA x2 + psB halves x4,
            # 2 v-tiles on psO), k-interleaved to match the x/wqk DMA pieces
            ps0 = psA.tile([128, NB], F32, tag="psA", name="ps")
            ps1 = psA.tile([128, NB], F32, tag="psA", name="ps")
            stb0 = psB.tile([128, 2 * QB], F32, tag="psB", name="st")
            stb1 = psB.tile([128, 2 * QB], F32, tag="psB", name="st")
            pv0 = psB.tile([128, HL * HD], F32, tag="psB", name="pv0")
            g_aps = [ps0[:], ps1[:], stb0[:, 0:512], stb0[:, 512:1024],
                     stb1[:, 0:512], stb1[:, 512:1024]]
            for ks in ((0, 1), (2, 3), (4, 5), (6, 7)):
                for g in range(6):
                    for k in ks:
                        nc.tensor.matmul(
                            g_aps[g],
                            lhsT=wqk_sb[k][:, g * 128:(g + 1) * 128],
                            rhs=xts[0][k],
                            start=(k == 0), stop=(k == KD - 1),
                        )
                s1_v_mm(0, 0, pv0[:], ks)
            # ropes interleaved with the remaining PE work (mo6/7, v2/3)
            s1_rope(0, 0, g_aps[0], on_act=True)
            s1_rope(0, 1, g_aps[1], on_act=True)
            s1_v_fin(0, 0, pv0, on_act=True)
            s1_rope(0, 2, g_aps[2], on_act=True)
            s1_qk(0, 6)
            s1_rope(0, 3, g_aps[3], on_act=True)
            s1_qk(0, 7)
            s1_rope(0, 4, g_aps[4], on_act=True)
            s1_v(0, 1, on_act=True)
            s1_rope(0, 5, g_aps[5], on_act=True)
            s1_v(0, 2, on_act=True)
            s1_v(0, 3, on_act=True)
            load_x(1, pieces=2)
            # qb=0: overlap with full s1(1)
            f0 = []
            for m in range(8):
                f0 += [lambda m=m: s1_qk_h(1, m, 0), lambda m=m: s1_qk_h(1, m, 1)]
                if m % 2 == 1:
                    f0.append(lambda m=m: s1_v(1, m // 2, on_act=True))
            run_qb(0, f0)
            load_x(2, pieces=2)
            # qb=1: overlap with full s1(2) + s3(0)
            f1 = []
            for m in range(8):
                f1 += [lambda m=m: s1_qk_h(2, m, 0), lambda m=m: s1_qk_h(2, m, 1)]
                if m % 2 == 1:
                    f1.append(lambda m=m: s1_v(2, m // 2))
                    f1.append(lambda m=m: s3_ib(0, m // 2, 0))
                    f1.append(lambda m=m: s3_ib(0, m // 2, 1))
            run_qb(1, f1)
            load_x(3, pieces=2)
            # qb=2: overlap with s1(3) q-projection + s3(1)
            f2 = []
            for m in range(4):
                f2 += [lambda m=m: s1_qk_h(3, m, 0), lambda m=m: s1_qk_h(3, m, 1)]
                f2 += [lambda m=m: s3_ib(1, m, 0), lambda m=m: s3_ib(1, m, 1)]
            run_qb(2, f2)
            # qb=3: backfill with deferred s1(3) k-projection + v + s3(2)
            f3 = [lambda: s1_qk_h(3, 4, 0), lambda: s1_qk_h(3, 4, 1)]
            f3 += [lambda m=m: s1_v(3, m) for m in range(4)]
            for m in (1, 2, 3):
                f3 += [lambda m=m: s1_qk_h(3, 4 + m, 0),
                       lambda m=m: s1_qk_h(3, 4 + m, 1)]
            f3 += [lambda m=m: s3_ib(2, m // 2, m % 2) for m in range(4)]
            # front-load k(hp0) + all v tiles so hp0's diagonal PVs are fed;
            # hold two s3(2) units back to cover the last tail chain
            run_qb(3, f3, slots=[1, 2, 4, 5, 7, 8, 12, 16, 20, 26, 32, 38,
                                 44, 50, 54, 58],
                   late=[lambda m=m: s3_mt(2, m) for m in (2, 3)])
            for mt in range(4):
                s3_mt(3, mt, alt=True)

    nc.compile()
    return nc


# ---------------------------------------------------------------------------
# host side
# ---------------------------------------------------------------------------

_cache = {}


def _get_nc(S):
    if S not in _cache:
        _cache[S] = build_nc(S)
    return _cache[S]


def _shard_weights(w_qkv, w_out, g):
    """Per-head-group weight shards in device layouts."""
    w_qkv = np.asarray(w_qkv, dtype=np.float32)
    w_out = np.asarray(w_out, dtype=np.float32)
    r = slice(g * 512, (g + 1) * 512)
    wq = w_qkv[0:D][r]            # [512, 1024]
    wk = w_qkv[D:2 * D][r]
    wv = w_qkv[2 * D:3 * D][r]
    wqkT = np.ascontiguousarray(np.concatenate([wq, wk], axis=0).T).astype(BF16_NP)
    wvT = np.ascontiguousarray(wv.T).astype(BF16_NP)                  # [1024, 512]
    woutT = np.ascontiguousarray(w_out.T[r]).astype(BF16_NP)          # [512, 1024]
    return wqkT, wvT, woutT


def kernel(x, w_qkv, w_out):
    x = np.asarray(x, dtype=np.float32)
    B, S, _D = x.shape
    assert _D == D
    nc = _get_nc(S)

    shards = [_shard_weights(w_qkv, w_out, g) for g in range(2)]
    in_maps = []
    for core in range(8):
        b, g = core // 2, core % 2
        wqkT, wvT, woutT = shards[g]
        in_maps.append({
            "xT": np.ascontiguousarray(x[b].T).astype(BF16_NP),
            "wqkT": wqkT,
            "wvT": wvT,
            "woutT": woutT,
        })
    res = run_bass_kernel_spmd(nc, in_maps, list(range(8)))
    out = np.empty((B, S, D), dtype=np.float32)
    for b in range(B):
        out[b] = (res.results[2 * b]["outp"].astype(np.float32)
                  + res.results[2 * b + 1]["outp"].astype(np.float32))
    return out


# revision 10
# speedup vs baseline: 1.0825x; 1.0306x over previous
"""Multi-head causal attention (B=4, S=2048, D=1024, H=16, RoPE) on 8 TRN2 cores.

Sharding: core = (batch b, head-group g of 8 heads).  Each core computes
qkv projection for its (b, g), RoPE, causal attention, and a partial
out-projection (contraction over its 512 head-dims).  Host sums the two
partials per batch.

Pipeline: qb-outer software pipeline; stage1(nb+1)/stage3(qb-1) units are
interleaved between stage2(qb) iterations as PE backfill (the j-loop is
exp/ACT-bound).  All matmuls run in bf16 (1 cyc/row at any free size).
RoPE rotate-half is a PE permutation matmul (cos/sin tables are invariant
under the 32-row swap).  The P@V matmul is emitted TRANSPOSED - output
[q-partitions, head-dims] - so all 128 output partitions are used (~2x
fewer PE cycles than [dims, q-free]) and the ones-column softmax
denominators land per-partition, where a reciprocal + tensor_scalar
normalize is cheap; a PE transpose then returns O^T to feature-major for
the out-projection.

Device layouts (per core):
  qk^T  [1024, S]  bf16 feature-major: tiles 0:4 = q (8 heads x 64), 4:8 = k
  v     [S, 520]   bf16 token-major, 65 cols/head: 64 dims + ones column
                   (accumulates softmax denominators during the P@V matmul)
  S^T   [t, q]     scores transposed; P tiles feed P@V as lhsT
  O^T   [512, S]   bf16 per-head outputs, feature-major, out-proj lhsT
  outp  [S, 1024]  fp16 partial (host sums the two cores of each batch)
"""

import math

import ml_dtypes
import numpy as np

import concourse.bacc as bacc
import concourse.mybir as mybir
from concourse import tile
from concourse.bass_utils import run_bass_kernel_spmd

AF = mybir.ActivationFunctionType
ALU = mybir.AluOpType
F32 = mybir.dt.float32
F32R = mybir.dt.float32r
BF16 = mybir.dt.bfloat16
BF16_NP = ml_dtypes.bfloat16

N_HEADS = 16
THETA = 10000.0
D = 1024
HD = 64
HL = 8          # heads per core
VW = HD + 1     # v columns per head (64 dims + ones)
NB = 512        # stage-1 token block
QB = 512        # query block
TT = 128        # key/value tile


def _host_constants(S):
    """RoPE tables, signed-swap permutation, causal mask (input-independent)."""
    half = HD // 2
    inv = 1.0 / (THETA ** (np.arange(half, dtype=np.float64) / half))
    t = np.arange(S, dtype=np.float64)
    ang = inv[:, None] * t[None, :]                      # [32, S]
    ropeC = np.tile(np.cos(ang), (4, 1)).astype(BF16_NP)   # [128, S]
    sinT = np.tile(np.sin(ang), (4, 1)).astype(BF16_NP)    # [128, S]

    # perm[k, m] = sig(m) iff k == swap(m): out[m] = sig(m) * in[swap(m)]
    perm = np.zeros((128, 128), dtype=BF16_NP)
    for m in range(128):
        blk, inner = (m // HD) * HD, m % HD
        partner = blk + (inner + half) % HD
        perm[partner, m] = -1.0 if inner < half else 1.0

    # maskT2[p, c] over two 128-col copies: upper-triangular keep (c >= p)
    p = np.arange(TT)[:, None]
    c = np.arange(TT)[None, :]
    m1 = (c >= p).astype(BF16_NP)
    maskT2 = np.concatenate([m1, m1], axis=1)            # [128, 256]
    return ropeC, sinT, perm, maskT2


def build_nc(S=2048):
    nc = bacc.Bacc("TRN2", target_bir_lowering=False, debug=False)

    xT = nc.dram_tensor("xT", [D, S], BF16, kind="ExternalInput").ap()
    wqkT = nc.dram_tensor("wqkT", [D, 2 * HL * HD], BF16, kind="ExternalInput").ap()
    wvT = nc.dram_tensor("wvT", [D, HL * HD], BF16, kind="ExternalInput").ap()
    woutT = nc.dram_tensor("woutT", [HL * HD, D], BF16, kind="ExternalInput").ap()
    outp = nc.dram_tensor("outp", [S, D], mybir.dt.float16, kind="ExternalOutput").ap()

    ropeC_np, sinT_np, perm_np, maskT2_np = _host_constants(S)
    ropeC_d = nc.inline_tensor(ropeC_np, "ropeC").ap()
    sinT_d = nc.inline_tensor(sinT_np, "sinT").ap()
    perm_d = nc.inline_tensor(perm_np, "perm").ap()
    maskT2_d = nc.inline_tensor(maskT2_np, "maskT2").ap()
    ident_d = nc.inline_tensor(np.eye(128, dtype=BF16_NP), "ident").ap()

    KD = D // 128        # 8 contraction tiles
    nNB = S // NB        # 4
    nQB = S // QB        # 4
    NTPB = QB // TT      # 4
    scale = 1.0 / math.sqrt(HD)
    PVLAG = 0

    with tile.TileContext(nc) as tc:
        with (
            tc.tile_pool(name="qk", bufs=1) as qk_pool,
            tc.tile_pool(name="vres", bufs=1) as v_pool,
            tc.tile_pool(name="osb", bufs=1) as o_pool,
            tc.tile_pool(name="wqk", bufs=1) as wqk_pool,
            tc.tile_pool(name="wv", bufs=1) as wv_pool,
            tc.tile_pool(name="wout", bufs=1) as wout_pool,
            tc.tile_pool(name="tabs", bufs=1) as tab_pool,
            tc.tile_pool(name="xs", bufs=2) as x_pool,
            tc.tile_pool(name="t1p", bufs=6) as t1_pool,
            tc.tile_pool(name="ptp", bufs=26) as pt_pool,
            tc.tile_pool(name="recp", bufs=3) as rec_pool,
            tc.tile_pool(name="t2p", bufs=4) as t2_pool,
            tc.tile_pool(name="otp", bufs=4) as ot_pool,
            tc.tile_pool(name="psA", bufs=2, space="PSUM") as psA,
            tc.tile_pool(name="psB", bufs=3, space="PSUM") as psB,
        ):
            qk_sb = [qk_pool.tile([128, S], BF16, tag=f"qk{i}", name=f"qk{i}")
                     for i in range(8)]
            v_sb = [v_pool.tile([128, HL * VW], BF16, tag=f"v{i}", name=f"v{i}")
                    for i in range(S // TT)]
            o_sb = [o_pool.tile([128, S], BF16, tag=f"o{i}", name=f"o{i}")
                    for i in range(4)]
            wqkb = wqk_pool.tile([128, KD * 2 * HL * HD], BF16, tag="wqkb", name="wqkb")
            wvb = wv_pool.tile([128, KD * HL * HD], BF16, tag="wvb", name="wvb")
            wob = wout_pool.tile([128, 4 * D], BF16, tag="wob", name="wob")
            wqk_sb = [wqkb[:, k * 2 * HL * HD:(k + 1) * 2 * HL * HD] for k in range(KD)]
            wv_sb = [wvb[:, k * HL * HD:(k + 1) * HL * HD] for k in range(KD)]
            wout_sb = [wob[:, i * D:(i + 1) * D] for i in range(4)]
            ropeC_sb = tab_pool.tile([128, S], BF16, tag="ropeC", name="ropeC")
            sinT_sb = tab_pool.tile([128, S], BF16, tag="sinT", name="sinT")
            perm_sb = tab_pool.tile([128, 128], BF16, tag="perm", name="perm")
            maskT2_sb = tab_pool.tile([128, 2 * TT], BF16, tag="maskT2", name="maskT2")
            ident_sb = tab_pool.tile([128, 128], BF16, tag="ident", name="ident")

            xts = {}   # nb -> list of 8 [128, NB] f32r column slices

            xT_r = xT.rearrange("(k p) c -> p k c", k=KD)

            def load_x(nb, queue=nc.sync, pieces=1):
                tk = slice(nb * NB, (nb + 1) * NB)
                xb = x_pool.tile([128, KD * NB], BF16, name="xb")
                xb_r = xb[:].rearrange("p (k c) -> p k c", k=KD)
                kstep = KD // pieces
                for i in range(pieces):
                    ksl = slice(i * kstep, (i + 1) * kstep)
                    queue.dma_start(xb_r[:, ksl, :], xT_r[:, ksl, tk])
                xts[nb] = [xb[:, k * NB:(k + 1) * NB] for k in range(KD)]

            # ---------------- preamble: weights + first x block ----------
            # split across issue queues so the first matmul group starts fast
            wqkT_r = wqkT.rearrange("(k p) m -> p k m", k=KD)
            wqkb_r = wqkb[:].rearrange("p (k m) -> p k m", k=KD)
            wvT_r = wvT.rearrange("(k p) m -> p k m", k=KD)
            wvb_r = wvb[:].rearrange("p (k m) -> p k m", k=KD)
            woutT_r = woutT.rearrange("(i p) m -> p i m", i=4)
            wob_r = wob[:].rearrange("p (i m) -> p i m", i=4)
            # interleave x / wqk pieces so the first k-chain starts early
            xb0 = x_pool.tile([128, KD * NB], BF16, name="xb")
            xb0_r = xb0[:].rearrange("p (k c) -> p k c", k=KD)
            nc.scalar.dma_start(ropeC_sb[:], ropeC_d[:])
            nc.scalar.dma_start(sinT_sb[:], sinT_d[:])
            nc.scalar.dma_start(perm_sb[:], perm_d[:])
            for k0, k1 in ((0, 1), (1, 2), (2, 4), (4, 8)):
                ksl = slice(k0, k1)
                nc.sync.dma_start(xb0_r[:, ksl, :], xT_r[:, ksl, 0:NB])
                nc.sync.dma_start(wqkb_r[:, ksl, :], wqkT_r[:, ksl, :])
            xts[0] = [xb0[:, k * NB:(k + 1) * NB] for k in range(KD)]
            nc.scalar.dma_start(wvb_r[:], wvT_r[:])
            nc.scalar.dma_start(maskT2_sb[:], maskT2_d[:])
            nc.scalar.dma_start(ident_sb[:], ident_d[:])
            nc.scalar.dma_start(wob_r[:], woutT_r[:])

            # ---------------- stage 1: qkv projection + RoPE --------------
            def s1_rope(nb, mo, ps_ap, on_act=True):
                tok = slice(nb * NB, (nb + 1) * NB)
                dst = qk_sb[mo][:, tok]
                qs = psA.tile([128, NB], F32, tag="psA", name="qs")
                if on_act:
                    # PSUM read on ACT; bf16 SBUF DVE ops run in 2x mode
                    praw = t1_pool.tile([128, NB], BF16, tag="t1", name="praw")
                    nc.scalar.copy(praw[:], ps_ap)
                    nc.tensor.matmul(qs[:], lhsT=perm_sb[:], rhs=praw[:],
                                     start=True, stop=True)
                    nc.vector.tensor_tensor(dst, praw[:], ropeC_sb[:, tok],
                                            op=ALU.mult)
                    tm = t1_pool.tile([128, NB], BF16, tag="t1", name="tm")
                    nc.vector.tensor_tensor(tm[:], qs[:], sinT_sb[:, tok],
                                            op=ALU.mult)
                    nc.vector.tensor_tensor(dst, dst, tm[:], op=ALU.add)
                else:
                    # DVE-only variant (sin table is swap-invariant)
                    t1 = t1_pool.tile([128, NB], BF16, tag="t1", name="t1")
                    nc.vector.tensor_tensor(t1[:], ps_ap, sinT_sb[:, tok],
                                            op=ALU.mult)
                    nc.tensor.matmul(qs[:], lhsT=perm_sb[:], rhs=t1[:],
                                     start=True, stop=True)
                    nc.vector.tensor_tensor(dst, ps_ap, ropeC_sb[:, tok],
                                            op=ALU.mult)
                    nc.vector.tensor_tensor(dst, dst, qs[:], op=ALU.add)

            _qk_ps = {}

            def s1_qk_h(nb, mo, half):
                if half == 0:
                    _qk_ps[(nb, mo)] = psA.tile([128, NB], F32, tag="psA",
                                                name="ps")
                ps = _qk_ps[(nb, mo)]
                for k in range(half * 4, half * 4 + 4):
                    nc.tensor.matmul(
                        ps[:],
                        lhsT=wqk_sb[k][:, mo * 128:(mo + 1) * 128],
                        rhs=xts[nb][k][:],
                        start=(k == 0), stop=(k == KD - 1),
                    )
                if half == 1:
                    del _qk_ps[(nb, mo)]
                    if nb == 1:
                        on_act = True
                    elif nb == 3:
                        on_act = False
                    else:
                        on_act = (mo % 2 == 0)
                    s1_rope(nb, mo, ps[:], on_act=on_act)

            def s1_qk(nb, mo):
                s1_qk_h(nb, mo, 0)
                s1_qk_h(nb, mo, 1)

            def s1_v_mm(nb, mt, pv_ap, ks):
                xsl = slice(mt * 128, (mt + 1) * 128)
                for k in ks:
                    nc.tensor.matmul(
                        pv_ap,
                        lhsT=xts[nb][k][:, xsl],
                        rhs=wv_sb[k],
                        start=(k == 0), stop=(k == KD - 1),
                    )

            def s1_v_fin(nb, mt, pv, on_act=False):
                vt = v_sb[nb * (NB // TT) + mt]
                vre = vt[:].rearrange("p (h c) -> p h c", h=HL)
                if on_act:
                    nc.scalar.copy(
                        vre[:, :, 0:HD], pv[:].rearrange("p (h c) -> p h c", h=HL))
                else:
                    nc.vector.tensor_copy(
                        vre[:, :, 0:HD], pv[:].rearrange("p (h c) -> p h c", h=HL))
                nc.vector.memset(vre[:, :, HD], 1.0)

            def s1_v(nb, mt, on_act=False):
                pv = psA.tile([128, HL * HD], F32, tag="psA", name="pv")
                s1_v_mm(nb, mt, pv[:], range(KD))
                s1_v_fin(nb, mt, pv, on_act=on_act)

            # ---------------- stage 2: causal attention ------------------
            def s2_scores(qb, hp, j, pts):
                qt = qk_sb[hp]
                kt = qk_sb[4 + hp]
                oi = j - NTPB * qb
                c0 = max(oi, 0) * TT
                st = psB.tile([128, 2 * QB], F32, tag="psB", name="st")
                for hi in (0, 1):
                    base = hi * HD
                    nc.tensor.matmul(
                        st[:, hi * QB + c0:(hi + 1) * QB],
                        lhsT=kt[base:base + HD, j * TT:(j + 1) * TT],
                        rhs=qt[base:base + HD, qb * QB + c0:(qb + 1) * QB],
                        start=True, stop=True,
                    )
                pt = pt_pool.tile([128, 2 * QB], BF16, name="pt")
                st2 = st[:].rearrange("p (h c) -> p h c", h=2)
                pt2 = pt[:].rearrange("p (h c) -> p h c", h=2)
                nc.scalar.activation(pt2[:, :, c0:QB], st2[:, :, c0:QB],
                                     AF.Exp, scale=scale)
                if oi >= 0:
                    eng = nc.vector
                    for hi in (0, 1):
                        csl = slice(hi * QB + c0, hi * QB + c0 + TT)
                        eng.tensor_tensor(
                            pt[:, csl], pt[:, csl],
                            maskT2_sb[:, hi * TT:(hi + 1) * TT],
                            op=ALU.mult)
                pts[(hp, j)] = pt

            def s2_pv_group(qb, hp, sub, pts, oaug8, rec, o_qm):
                # full accumulation chain for q-subblock `sub` (both heads),
                # followed immediately by its reciprocal + normalize
                h0, h1 = 2 * hp, 2 * hp + 1
                jlast = NTPB * qb + sub
                for hi, hh in ((0, h0), (1, h1)):
                    g = hi * NTPB + sub
                    for j in range(jlast + 1):
                        nc.tensor.matmul(
                            oaug8[:, TT * g:TT * g + VW],
                            lhsT=pts[(hp, j)][:, hi * QB + sub * TT:hi * QB + (sub + 1) * TT],
                            rhs=v_sb[j][:, VW * hh:VW * hh + VW],
                            start=(j == 0), stop=(j == jlast),
                        )
                for hi in (0, 1):
                    g = hi * NTPB + sub
                    nc.vector.reciprocal(
                        rec[:, g:g + 1],
                        oaug8[:, TT * g + HD:TT * g + VW])
                    nc.vector.tensor_scalar_mul(
                        o_qm[:, sub * TT + hi * HD:sub * TT + hi * HD + HD],
                        oaug8[:, TT * g:TT * g + HD],
                        rec[:, g:g + 1])

            def s2_tail(qb, hp, o_qm):
                # PE transpose back to dim-major + one copy out
                tp = psA.tile([128, QB], BF16, tag="psA", name="tp")
                for sub in range(NTPB):
                    nc.tensor.transpose(
                        tp[:, sub * TT:(sub + 1) * TT],
                        o_qm[:, sub * TT:(sub + 1) * TT], ident_sb[:])
                qsl = slice(qb * QB, (qb + 1) * QB)
                if qb < 2:
                    nc.scalar.copy(o_sb[hp][:, qsl], tp[:])
                else:
                    nc.vector.tensor_copy(o_sb[hp][:, qsl], tp[:])

            def run_qb(qb, fillers, slots=None, late=()):
                """Emit one qb phase: lag-2 PV issue + filler interleave."""
                jmax = NTPB * (qb + 1)
                total = 4 * jmax
                if slots is None:
                    slots = [int(round((i + 1) * total / (len(fillers) + 1)))
                             for i in range(len(fillers))]
                fi = 0
                slot = 0
                prewarm = (3, 3, 3, 3)[qb]
                for hp in range(4):
                    oaug8 = psB.tile([128, 2 * QB], F32, tag="psB",
                                     name="oaug")
                    rec = rec_pool.tile([128, 2 * NTPB], F32, tag="rec8",
                                        name="rec")
                    o_qm = t2_pool.tile([128, QB], BF16, tag="t2", name="oqm")
                    start_j = prewarm if hp > 0 else 0
                    # groups whose diagonal falls before start_j (their
                    # scores were prewarmed) must still be emitted
                    for oi in range(0, start_j - NTPB * qb):
                        s2_pv_group(qb, hp, oi, pend_pts, oaug8, rec, o_qm)
                    for j in range(start_j, jmax):
                        s2_scores(qb, hp, j, pend_pts)
                        oi = j - NTPB * qb - PVLAG
                        if oi >= 0:
                            s2_pv_group(qb, hp, oi, pend_pts, oaug8, rec, o_qm)
                        slot += 1
                        while fi < len(fillers) and slots[fi] <= slot:
                            fillers[fi]()
                            fi += 1
                    for oi in range(NTPB - PVLAG, NTPB):
                        s2_pv_group(qb, hp, oi, pend_pts, oaug8, rec, o_qm)
                    # prewarm the next hp's first scores/exps: ACT chews them
                    # while this hp's PV chains and tail run on PE/DVE
                    if hp < 3:
                        for j2 in range(prewarm):
                            s2_scores(qb, hp + 1, j2, pend_pts)
                    for k in [k for k in pend_pts if k[0] == hp]:
                        del pend_pts[k]
                    s2_tail(qb, hp, o_qm)
                    if hp == 3:
                        for f in late:
                            f()
                while fi < len(fillers):
                    fillers[fi]()
                    fi += 1

            pend_pts = {}

            # ---------------- stage 3: out projection ---------------------
            _s3_ot = {}

            def s3_ib(qb, mt, ib, alt=False):
                    tsl = slice(qb * QB + mt * 128, qb * QB + (mt + 1) * 128)
                    if ib == 0:
                        _s3_ot[(qb, mt)] = ot_pool.tile(
                            [128, D], mybir.dt.float16, name="ot")
                    ot = _s3_ot[(qb, mt)]
                    po = psA.tile([128, 512], F32, tag="psA", name="po")
                    for hp in range(4):
                        nc.tensor.matmul(
                            po[:],
                            lhsT=o_sb[hp][:, tsl],
                            rhs=wout_sb[hp][:, ib * 512:(ib + 1) * 512],
                            start=(hp == 0), stop=(hp == 3),
                        )
                    osl = slice(ib * 512, (ib + 1) * 512)
                    if alt and ib == 0:
                        nc.scalar.copy(ot[:, osl], po[:])
                    else:
                        nc.vector.tensor_copy(ot[:, osl], po[:])
                    if ib == 1:
                        del _s3_ot[(qb, mt)]
                        nc.sync.dma_start(outp[tsl, :], ot[:])

            def s3_mt(qb, mt, alt=False):
                    s3_ib(qb, mt, 0, alt)
                    s3_ib(qb, mt, 1, alt)

            # ---------------- pipelined emission --------------------------
            # nb=0: 8 concurrent groups (6 qk-mo on psA x2 + psB halves x4,
            # 2 v-tiles on psO), k-interleaved to match the x/wqk DMA pieces
            ps0 = psA.tile([128, NB], F32, tag="psA", name="ps")
            ps1 = psA.tile([128, NB], F32, tag="psA", name="ps")
            stb0 = psB.tile([128, 2 * QB], F32, tag="psB", name="st")
            stb1 = psB.tile([128, 2 * QB], F32, tag="psB", name="st")
            pv0 = psB.tile([128, HL * HD], F32, tag="psB", name="pv0")
            g_aps = [ps0[:], ps1[:], stb0[:, 0:512], stb0[:, 512:1024],
                     stb1[:, 0:512], stb1[:, 512:1024]]
            for ks in ((0, 1), (2, 3), (4, 5), (6, 7)):
                for g in range(6):
                    for k in ks:
                        nc.tensor.matmul(
                            g_aps[g],
                            lhsT=wqk_sb[k][:, g * 128:(g + 1) * 128],
                            rhs=xts[0][k],
                            start=(k == 0), stop=(k == KD - 1),
                        )
                s1_v_mm(0, 0, pv0[:], ks)
            # ropes interleaved with the remaining PE work (mo6/7, v2/3)
            s1_rope(0, 0, g_aps[0], on_act=True)
            s1_rope(0, 1, g_aps[1], on_act=True)
            s1_v_fin(0, 0, pv0, on_act=True)
            s1_rope(0, 2, g_aps[2], on_act=True)
            s1_qk(0, 6)
            s1_rope(0, 3, g_aps[3], on_act=True)
            s1_qk(0, 7)
            s1_rope(0, 4, g_aps[4], on_act=True)
            s1_v(0, 1, on_act=True)
            s1_rope(0, 5, g_aps[5], on_act=True)
            s1_v(0, 2, on_act=True)
            s1_v(0, 3, on_act=True)
            load_x(1, pieces=2)
            # qb=0: overlap with full s1(1)
            f0 = []
            for m in range(8):
                f0 += [lambda m=m: s1_qk_h(1, m, 0), lambda m=m: s1_qk_h(1, m, 1)]
                if m % 2 == 1:
                    f0.append(lambda m=m: s1_v(1, m // 2, on_act=True))
            run_qb(0, f0)
            load_x(2, pieces=2)
            # qb=1: overlap with full s1(2) + s3(0)
            f1 = []
            for m in range(8):
                f1 += [lambda m=m: s1_qk_h(2, m, 0), lambda m=m: s1_qk_h(2, m, 1)]
                if m % 2 == 1:
                    f1.append(lambda m=m: s1_v(2, m // 2))
                    f1.append(lambda m=m: s3_ib(0, m // 2, 0))
                    f1.append(lambda m=m: s3_ib(0, m // 2, 1))
            run_qb(1, f1)
            load_x(3, pieces=2)
            # qb=2: overlap with s1(3) q-projection + s3(1)
            f2 = []
            for m in range(4):
                f2 += [lambda m=m: s1_qk_h(3, m, 0), lambda m=m: s1_qk_h(3, m, 1)]
                f2 += [lambda m=m: s3_ib(1, m, 0), lambda m=m: s3_ib(1, m, 1)]
            run_qb(2, f2)
            # qb=3: backfill with deferred s1(3) k-projection + v + s3(2)
            f3 = [lambda: s1_qk_h(3, 4, 0), lambda: s1_qk_h(3, 4, 1)]
            f3 += [lambda m=m: s1_v(3, m) for m in range(4)]
            for m in (1, 2, 3):
                f3 += [lambda m=m: s1_qk_h(3, 4 + m, 0),
                       lambda m=m: s1_qk_h(3, 4 + m, 1)]
            f3 += [lambda m=m: s3_ib(2, m // 2, m % 2) for m in range(4)]
            # front-load k(hp0) + all v tiles so hp0's diagonal PVs are fed;
            # hold two s3(2) units back to cover the last tail chain
            run_qb(3, f3, slots=[1, 2, 4, 5, 7, 8, 12, 16, 20, 26, 32, 38,
                                 44, 50, 54, 58],
                   late=[lambda m=m: s3_mt(2, m) for m in (2, 3)])
            for mt in range(4):
                s3_mt(3, mt, alt=True)

    nc.compile()
    return nc


# ---------------------------------------------------------------------------
# host side
# ---------------------------------------------------------------------------

_cache = {}


def _get_nc(S):
    if S not in _cache:
        _cache[S] = build_nc(S)
    return _cache[S]


def _shard_weights(w_qkv, w_out, g):
    """Per-head-group weight shards in device layouts."""
    w_qkv = np.asarray(w_qkv, dtype=np.float32)
    w_out = np.asarray(w_out, dtype=np.float32)
    r = slice(g * 512, (g + 1) * 512)
    wq = w_qkv[0:D][r]            # [512, 1024]
    wk = w_qkv[D:2 * D][r]
    wv = w_qkv[2 * D:3 * D][r]
    wqkT = np.ascontiguousarray(np.concatenate([wq, wk], axis=0).T).astype(BF16_NP)
    wvT = np.ascontiguousarray(wv.T).astype(BF16_NP)                  # [1024, 512]
    woutT = np.ascontiguousarray(w_out.T[r]).astype(BF16_NP)          # [512, 1024]
    return wqkT, wvT, woutT


def kernel(x, w_qkv, w_out):
    x = np.asarray(x, dtype=np.float32)
    B, S, _D = x.shape
    assert _D == D
    nc = _get_nc(S)

    shards = [_shard_weights(w_qkv, w_out, g) for g in range(2)]
    in_maps = []
    for core in range(8):
        b, g = core // 2, core % 2
        wqkT, wvT, woutT = shards[g]
        in_maps.append({
            "xT": np.ascontiguousarray(x[b].T).astype(BF16_NP),
            "wqkT": wqkT,
            "wvT": wvT,
            "woutT": woutT,
        })
    res = run_bass_kernel_spmd(nc, in_maps, list(range(8)))
    out = np.empty((B, S, D), dtype=np.float32)
    for b in range(B):
        out[b] = (res.results[2 * b]["outp"].astype(np.float32)
                  + res.results[2 * b + 1]["outp"].astype(np.float32))
    return out


# revision 11
# speedup vs baseline: 1.0828x; 1.0003x over previous
"""Multi-head causal attention (B=4, S=2048, D=1024, H=16, RoPE) on 8 TRN2 cores.

Sharding: core = (batch b, head-group g of 8 heads).  Each core computes
qkv projection for its (b, g), RoPE, causal attention, and a partial
out-projection (contraction over its 512 head-dims).  Host sums the two
partials per batch.

Pipeline: qb-outer software pipeline; stage1(nb+1)/stage3(qb-1) units are
interleaved between stage2(qb) iterations as PE backfill (the j-loop is
exp/ACT-bound).  All matmuls run in bf16 (1 cyc/row at any free size).
RoPE rotate-half is a PE permutation matmul (cos/sin tables are invariant
under the 32-row swap).  The P@V matmul is emitted TRANSPOSED - output
[q-partitions, head-dims] - so all 128 output partitions are used (~2x
fewer PE cycles than [dims, q-free]) and the ones-column softmax
denominators land per-partition, where a reciprocal + tensor_scalar
normalize is cheap; a PE transpose then returns O^T to feature-major for
the out-projection.

Device layouts (per core):
  qk^T  [1024, S]  bf16 feature-major: tiles 0:4 = q (8 heads x 64), 4:8 = k
  v     [S, 520]   bf16 token-major, 65 cols/head: 64 dims + ones column
                   (accumulates softmax denominators during the P@V matmul)
  S^T   [t, q]     scores transposed; P tiles feed P@V as lhsT
  O^T   [512, S]   bf16 per-head outputs, feature-major, out-proj lhsT
  outp  [S, 1024]  fp16 partial (host sums the two cores of each batch)
"""

import math

import ml_dtypes
import numpy as np

import concourse.bacc as bacc
import concourse.mybir as mybir
from concourse import tile
from concourse.bass_utils import run_bass_kernel_spmd

AF = mybir.ActivationFunctionType
ALU = mybir.AluOpType
F32 = mybir.dt.float32
F32R = mybir.dt.float32r
BF16 = mybir.dt.bfloat16
BF16_NP = ml_dtypes.bfloat16

N_HEADS = 16
THETA = 10000.0
D = 1024
HD = 64
HL = 8          # heads per core
VW = HD + 1     # v columns per head (64 dims + ones)
NB = 512        # stage-1 token block
QB = 512        # query block
TT = 128        # key/value tile


def _host_constants(S):
    """RoPE tables, signed-swap permutation, causal mask (input-independent)."""
    half = HD // 2
    inv = 1.0 / (THETA ** (np.arange(half, dtype=np.float64) / half))
    t = np.arange(S, dtype=np.float64)
    ang = inv[:, None] * t[None, :]                      # [32, S]
    ropeC = np.tile(np.cos(ang), (4, 1)).astype(BF16_NP)   # [128, S]
    sinT = np.tile(np.sin(ang), (4, 1)).astype(BF16_NP)    # [128, S]

    # perm[k, m] = sig(m) iff k == swap(m): out[m] = sig(m) * in[swap(m)]
    perm = np.zeros((128, 128), dtype=BF16_NP)
    for m in range(128):
        blk, inner = (m // HD) * HD, m % HD
        partner = blk + (inner + half) % HD
        perm[partner, m] = -1.0 if inner < half else 1.0

    # maskT2[p, c] over two 128-col copies: upper-triangular keep (c >= p)
    p = np.arange(TT)[:, None]
    c = np.arange(TT)[None, :]
    m1 = (c >= p).astype(BF16_NP)
    maskT2 = np.concatenate([m1, m1], axis=1)            # [128, 256]
    return ropeC, sinT, perm, maskT2


def build_nc(S=2048):
    nc = bacc.Bacc("TRN2", target_bir_lowering=False, debug=False)

    xT = nc.dram_tensor("xT", [D, S], BF16, kind="ExternalInput").ap()
    wqkT = nc.dram_tensor("wqkT", [D, 2 * HL * HD], BF16, kind="ExternalInput").ap()
    wvT = nc.dram_tensor("wvT", [D, HL * HD], BF16, kind="ExternalInput").ap()
    woutT = nc.dram_tensor("woutT", [HL * HD, D], BF16, kind="ExternalInput").ap()
    outp = nc.dram_tensor("outp", [S, D], mybir.dt.float16, kind="ExternalOutput").ap()

    ropeC_np, sinT_np, perm_np, maskT2_np = _host_constants(S)
    ropeC_d = nc.inline_tensor(ropeC_np, "ropeC").ap()
    sinT_d = nc.inline_tensor(sinT_np, "sinT").ap()
    perm_d = nc.inline_tensor(perm_np, "perm").ap()
    maskT2_d = nc.inline_tensor(maskT2_np, "maskT2").ap()
    ident_d = nc.inline_tensor(np.eye(128, dtype=BF16_NP), "ident").ap()

    KD = D // 128        # 8 contraction tiles
    nNB = S // NB        # 4
    nQB = S // QB        # 4
    NTPB = QB // TT      # 4
    scale = 1.0 / math.sqrt(HD)
    PVLAG = 0

    with tile.TileContext(nc) as tc:
        with (
            tc.tile_pool(name="qk", bufs=1) as qk_pool,
            tc.tile_pool(name="vres", bufs=1) as v_pool,
            tc.tile_pool(name="osb", bufs=1) as o_pool,
            tc.tile_pool(name="wqk", bufs=1) as wqk_pool,
            tc.tile_pool(name="wv", bufs=1) as wv_pool,
            tc.tile_pool(name="wout", bufs=1) as wout_pool,
            tc.tile_pool(name="tabs", bufs=1) as tab_pool,
            tc.tile_pool(name="xs", bufs=2) as x_pool,
            tc.tile_pool(name="t1p", bufs=8) as t1_pool,
            tc.tile_pool(name="ptp", bufs=26) as pt_pool,
            tc.tile_pool(name="recp", bufs=4) as rec_pool,
            tc.tile_pool(name="t2p", bufs=6) as t2_pool,
            tc.tile_pool(name="otp", bufs=6) as ot_pool,
            tc.tile_pool(name="psA", bufs=2, space="PSUM") as psA,
            tc.tile_pool(name="psB", bufs=3, space="PSUM") as psB,
        ):
            qk_sb = [qk_pool.tile([128, S], BF16, tag=f"qk{i}", name=f"qk{i}")
                     for i in range(8)]
            v_sb = [v_pool.tile([128, HL * VW], BF16, tag=f"v{i}", name=f"v{i}")
                    for i in range(S // TT)]
            o_sb = [o_pool.tile([128, S], BF16, tag=f"o{i}", name=f"o{i}")
                    for i in range(4)]
            wqkb = wqk_pool.tile([128, KD * 2 * HL * HD], BF16, tag="wqkb", name="wqkb")
            wvb = wv_pool.tile([128, KD * HL * HD], BF16, tag="wvb", name="wvb")
            wob = wout_pool.tile([128, 4 * D], BF16, tag="wob", name="wob")
            wqk_sb = [wqkb[:, k * 2 * HL * HD:(k + 1) * 2 * HL * HD] for k in range(KD)]
            wv_sb = [wvb[:, k * HL * HD:(k + 1) * HL * HD] for k in range(KD)]
            wout_sb = [wob[:, i * D:(i + 1) * D] for i in range(4)]
            ropeC_sb = tab_pool.tile([128, S], BF16, tag="ropeC", name="ropeC")
            sinT_sb = tab_pool.tile([128, S], BF16, tag="sinT", name="sinT")
            perm_sb = tab_pool.tile([128, 128], BF16, tag="perm", name="perm")
            maskT2_sb = tab_pool.tile([128, 2 * TT], BF16, tag="maskT2", name="maskT2")
            ident_sb = tab_pool.tile([128, 128], BF16, tag="ident", name="ident")

            xts = {}   # nb -> list of 8 [128, NB] f32r column slices

            xT_r = xT.rearrange("(k p) c -> p k c", k=KD)

            def load_x(nb, queue=nc.sync, pieces=1):
                tk = slice(nb * NB, (nb + 1) * NB)
                xb = x_pool.tile([128, KD * NB], BF16, name="xb")
                xb_r = xb[:].rearrange("p (k c) -> p k c", k=KD)
                kstep = KD // pieces
                for i in range(pieces):
                    ksl = slice(i * kstep, (i + 1) * kstep)
                    queue.dma_start(xb_r[:, ksl, :], xT_r[:, ksl, tk])
                xts[nb] = [xb[:, k * NB:(k + 1) * NB] for k in range(KD)]

            # ---------------- preamble: weights + first x block ----------
            # split across issue queues so the first matmul group starts fast
            wqkT_r = wqkT.rearrange("(k p) m -> p k m", k=KD)
            wqkb_r = wqkb[:].rearrange("p (k m) -> p k m", k=KD)
            wvT_r = wvT.rearrange("(k p) m -> p k m", k=KD)
            wvb_r = wvb[:].rearrange("p (k m) -> p k m", k=KD)
            woutT_r = woutT.rearrange("(i p) m -> p i m", i=4)
            wob_r = wob[:].rearrange("p (i m) -> p i m", i=4)
            # interleave x / wqk pieces so the first k-chain starts early
            xb0 = x_pool.tile([128, KD * NB], BF16, name="xb")
            xb0_r = xb0[:].rearrange("p (k c) -> p k c", k=KD)
            nc.scalar.dma_start(ropeC_sb[:], ropeC_d[:])
            nc.scalar.dma_start(sinT_sb[:], sinT_d[:])
            nc.scalar.dma_start(perm_sb[:], perm_d[:])
            for k0, k1 in ((0, 1), (1, 2), (2, 4), (4, 8)):
                ksl = slice(k0, k1)
                nc.sync.dma_start(xb0_r[:, ksl, :], xT_r[:, ksl, 0:NB])
                nc.sync.dma_start(wqkb_r[:, ksl, :], wqkT_r[:, ksl, :])
            xts[0] = [xb0[:, k * NB:(k + 1) * NB] for k in range(KD)]
            nc.scalar.dma_start(wvb_r[:], wvT_r[:])
            nc.scalar.dma_start(maskT2_sb[:], maskT2_d[:])
            nc.scalar.dma_start(ident_sb[:], ident_d[:])
            nc.scalar.dma_start(wob_r[:], woutT_r[:])

            # ---------------- stage 1: qkv projection + RoPE --------------
            def s1_rope(nb, mo, ps_ap, on_act=True):
                tok = slice(nb * NB, (nb + 1) * NB)
                dst = qk_sb[mo][:, tok]
                qs = psA.tile([128, NB], F32, tag="psA", name="qs")
                if on_act:
                    # PSUM read on ACT; bf16 SBUF DVE ops run in 2x mode
                    praw = t1_pool.tile([128, NB], BF16, tag="t1", name="praw")
                    nc.scalar.copy(praw[:], ps_ap)
                    nc.tensor.matmul(qs[:], lhsT=perm_sb[:], rhs=praw[:],
                                     start=True, stop=True)
                    nc.vector.tensor_tensor(dst, praw[:], ropeC_sb[:, tok],
                                            op=ALU.mult)
                    tm = t1_pool.tile([128, NB], BF16, tag="t1", name="tm")
                    nc.vector.tensor_tensor(tm[:], qs[:], sinT_sb[:, tok],
                                            op=ALU.mult)
                    nc.vector.tensor_tensor(dst, dst, tm[:], op=ALU.add)
                else:
                    # DVE-only variant (sin table is swap-invariant)
                    t1 = t1_pool.tile([128, NB], BF16, tag="t1", name="t1")
                    nc.vector.tensor_tensor(t1[:], ps_ap, sinT_sb[:, tok],
                                            op=ALU.mult)
                    nc.tensor.matmul(qs[:], lhsT=perm_sb[:], rhs=t1[:],
                                     start=True, stop=True)
                    nc.vector.tensor_tensor(dst, ps_ap, ropeC_sb[:, tok],
                                            op=ALU.mult)
                    nc.vector.tensor_tensor(dst, dst, qs[:], op=ALU.add)

            _qk_ps = {}

            def s1_qk_h(nb, mo, half):
                if half == 0:
                    _qk_ps[(nb, mo)] = psA.tile([128, NB], F32, tag="psA",
                                                name="ps")
                ps = _qk_ps[(nb, mo)]
                for k in range(half * 4, half * 4 + 4):
                    nc.tensor.matmul(
                        ps[:],
                        lhsT=wqk_sb[k][:, mo * 128:(mo + 1) * 128],
                        rhs=xts[nb][k][:],
                        start=(k == 0), stop=(k == KD - 1),
                    )
                if half == 1:
                    del _qk_ps[(nb, mo)]
                    if nb == 1:
                        on_act = True
                    elif nb == 3:
                        on_act = False
                    else:
                        on_act = (mo % 2 == 0)
                    s1_rope(nb, mo, ps[:], on_act=on_act)

            def s1_qk(nb, mo):
                s1_qk_h(nb, mo, 0)
                s1_qk_h(nb, mo, 1)

            def s1_v_mm(nb, mt, pv_ap, ks):
                xsl = slice(mt * 128, (mt + 1) * 128)
                for k in ks:
                    nc.tensor.matmul(
                        pv_ap,
                        lhsT=xts[nb][k][:, xsl],
                        rhs=wv_sb[k],
                        start=(k == 0), stop=(k == KD - 1),
                    )

            def s1_v_fin(nb, mt, pv, on_act=False):
                vt = v_sb[nb * (NB // TT) + mt]
                vre = vt[:].rearrange("p (h c) -> p h c", h=HL)
                if on_act:
                    nc.scalar.copy(
                        vre[:, :, 0:HD], pv[:].rearrange("p (h c) -> p h c", h=HL))
                else:
                    nc.vector.tensor_copy(
                        vre[:, :, 0:HD], pv[:].rearrange("p (h c) -> p h c", h=HL))
                nc.vector.memset(vre[:, :, HD], 1.0)

            def s1_v(nb, mt, on_act=False):
                pv = psA.tile([128, HL * HD], F32, tag="psA", name="pv")
                s1_v_mm(nb, mt, pv[:], range(KD))
                s1_v_fin(nb, mt, pv, on_act=on_act)

            # ---------------- stage 2: causal attention ------------------
            def s2_scores(qb, hp, j, pts):
                qt = qk_sb[hp]
                kt = qk_sb[4 + hp]
                oi = j - NTPB * qb
                c0 = max(oi, 0) * TT
                st = psB.tile([128, 2 * QB], F32, tag="psB", name="st")
                for hi in (0, 1):
                    base = hi * HD
                    nc.tensor.matmul(
                        st[:, hi * QB + c0:(hi + 1) * QB],
                        lhsT=kt[base:base + HD, j * TT:(j + 1) * TT],
                        rhs=qt[base:base + HD, qb * QB + c0:(qb + 1) * QB],
                        start=True, stop=True,
                    )
                pt = pt_pool.tile([128, 2 * QB], BF16, name="pt")
                st2 = st[:].rearrange("p (h c) -> p h c", h=2)
                pt2 = pt[:].rearrange("p (h c) -> p h c", h=2)
                nc.scalar.activation(pt2[:, :, c0:QB], st2[:, :, c0:QB],
                                     AF.Exp, scale=scale)
                if oi >= 0:
                    eng = nc.vector
                    for hi in (0, 1):
                        csl = slice(hi * QB + c0, hi * QB + c0 + TT)
                        eng.tensor_tensor(
                            pt[:, csl], pt[:, csl],
                            maskT2_sb[:, hi * TT:(hi + 1) * TT],
                            op=ALU.mult)
                pts[(hp, j)] = pt

            def s2_pv_group(qb, hp, sub, pts, oaug8, rec, o_qm):
                # full accumulation chain for q-subblock `sub` (both heads),
                # followed immediately by its reciprocal + normalize
                h0, h1 = 2 * hp, 2 * hp + 1
                jlast = NTPB * qb + sub
                for hi, hh in ((0, h0), (1, h1)):
                    g = hi * NTPB + sub
                    for j in range(jlast + 1):
                        nc.tensor.matmul(
                            oaug8[:, TT * g:TT * g + VW],
                            lhsT=pts[(hp, j)][:, hi * QB + sub * TT:hi * QB + (sub + 1) * TT],
                            rhs=v_sb[j][:, VW * hh:VW * hh + VW],
                            start=(j == 0), stop=(j == jlast),
                        )
                for hi in (0, 1):
                    g = hi * NTPB + sub
                    nc.vector.reciprocal(
                        rec[:, g:g + 1],
                        oaug8[:, TT * g + HD:TT * g + VW])
                    nc.vector.tensor_scalar_mul(
                        o_qm[:, sub * TT + hi * HD:sub * TT + hi * HD + HD],
                        oaug8[:, TT * g:TT * g + HD],
                        rec[:, g:g + 1])

            def s2_tail(qb, hp, o_qm):
                # PE transpose back to dim-major + one copy out
                tp = psA.tile([128, QB], BF16, tag="psA", name="tp")
                for sub in range(NTPB):
                    nc.tensor.transpose(
                        tp[:, sub * TT:(sub + 1) * TT],
                        o_qm[:, sub * TT:(sub + 1) * TT], ident_sb[:])
                qsl = slice(qb * QB, (qb + 1) * QB)
                if qb < 2:
                    nc.scalar.copy(o_sb[hp][:, qsl], tp[:])
                else:
                    nc.vector.tensor_copy(o_sb[hp][:, qsl], tp[:])

            def run_qb(qb, fillers, slots=None, late=()):
                """Emit one qb phase: lag-2 PV issue + filler interleave."""
                jmax = NTPB * (qb + 1)
                total = 4 * jmax
                if slots is None:
                    slots = [int(round((i + 1) * total / (len(fillers) + 1)))
                             for i in range(len(fillers))]
                fi = 0
                slot = 0
                prewarm = (3, 3, 3, 3)[qb]
                for hp in range(4):
                    oaug8 = psB.tile([128, 2 * QB], F32, tag="psB",
                                     name="oaug")
                    rec = rec_pool.tile([128, 2 * NTPB], F32, tag="rec8",
                                        name="rec")
                    o_qm = t2_pool.tile([128, QB], BF16, tag="t2", name="oqm")
                    start_j = prewarm if hp > 0 else 0
                    # groups whose diagonal falls before start_j (their
                    # scores were prewarmed) must still be emitted
                    for oi in range(0, start_j - NTPB * qb):
                        s2_pv_group(qb, hp, oi, pend_pts, oaug8, rec, o_qm)
                    for j in range(start_j, jmax):
                        s2_scores(qb, hp, j, pend_pts)
                        oi = j - NTPB * qb - PVLAG
                        if oi >= 0:
                            s2_pv_group(qb, hp, oi, pend_pts, oaug8, rec, o_qm)
                        slot += 1
                        while fi < len(fillers) and slots[fi] <= slot:
                            fillers[fi]()
                            fi += 1
                    for oi in range(NTPB - PVLAG, NTPB):
                        s2_pv_group(qb, hp, oi, pend_pts, oaug8, rec, o_qm)
                    # prewarm the next hp's first scores/exps: ACT chews them
                    # while this hp's PV chains and tail run on PE/DVE
                    if hp < 3:
                        for j2 in range(prewarm):
                            s2_scores(qb, hp + 1, j2, pend_pts)
                    for k in [k for k in pend_pts if k[0] == hp]:
                        del pend_pts[k]
                    s2_tail(qb, hp, o_qm)
                    if hp == 3:
                        for f in late:
                            f()
                while fi < len(fillers):
                    fillers[fi]()
                    fi += 1

            pend_pts = {}

            # ---------------- stage 3: out projection ---------------------
            _s3_ot = {}

            def s3_ib(qb, mt, ib, alt=False):
                    tsl = slice(qb * QB + mt * 128, qb * QB + (mt + 1) * 128)
                    if ib == 0:
                        _s3_ot[(qb, mt)] = ot_pool.tile(
                            [128, D], mybir.dt.float16, name="ot")
                    ot = _s3_ot[(qb, mt)]
                    po = psA.tile([128, 512], F32, tag="psA", name="po")
                    for hp in range(4):
                        nc.tensor.matmul(
                            po[:],
                            lhsT=o_sb[hp][:, tsl],
                            rhs=wout_sb[hp][:, ib * 512:(ib + 1) * 512],
                            start=(hp == 0), stop=(hp == 3),
                        )
                    osl = slice(ib * 512, (ib + 1) * 512)
                    if alt and ib == 0:
                        nc.scalar.copy(ot[:, osl], po[:])
                    else:
                        nc.vector.tensor_copy(ot[:, osl], po[:])
                    if ib == 1:
                        del _s3_ot[(qb, mt)]
                        nc.sync.dma_start(outp[tsl, :], ot[:])

            def s3_mt(qb, mt, alt=False):
                    s3_ib(qb, mt, 0, alt)
                    s3_ib(qb, mt, 1, alt)

            # ---------------- pipelined emission --------------------------
            # nb=0: 8 concurrent groups (6 qk-mo on psA x2 + psB halves x4,
            # 2 v-tiles on psO), k-interleaved to match the x/wqk DMA pieces
            ps0 = psA.tile([128, NB], F32, tag="psA", name="ps")
            ps1 = psA.tile([128, NB], F32, tag="psA", name="ps")
            stb0 = psB.tile([128, 2 * QB], F32, tag="psB", name="st")
            stb1 = psB.tile([128, 2 * QB], F32, tag="psB", name="st")
            pv0 = psB.tile([128, HL * HD], F32, tag="psB", name="pv0")
            g_aps = [ps0[:], ps1[:], stb0[:, 0:512], stb0[:, 512:1024],
                     stb1[:, 0:512], stb1[:, 512:1024]]
            for ks in ((0, 1), (2, 3), (4, 5), (6, 7)):
                for g in range(6):
                    for k in ks:
                        nc.tensor.matmul(
                            g_aps[g],
                            lhsT=wqk_sb[k][:, g * 128:(g + 1) * 128],
                            rhs=xts[0][k],
                            start=(k == 0), stop=(k == KD - 1),
                        )
                s1_v_mm(0, 0, pv0[:], ks)
            # ropes interleaved with the remaining PE work (mo6/7, v2/3)
            s1_rope(0, 0, g_aps[0], on_act=True)
            s1_rope(0, 1, g_aps[1], on_act=True)
            s1_v_fin(0, 0, pv0, on_act=True)
            s1_rope(0, 2, g_aps[2], on_act=True)
            s1_qk(0, 6)
            s1_rope(0, 3, g_aps[3], on_act=True)
            s1_qk(0, 7)
            s1_rope(0, 4, g_aps[4], on_act=True)
            s1_v(0, 1, on_act=True)
            s1_rope(0, 5, g_aps[5], on_act=True)
            s1_v(0, 2, on_act=True)
            s1_v(0, 3, on_act=True)
            load_x(1, pieces=2)
            # qb=0: overlap with full s1(1)
            f0 = []
            for m in range(8):
                f0 += [lambda m=m: s1_qk_h(1, m, 0), lambda m=m: s1_qk_h(1, m, 1)]
                if m % 2 == 1:
                    f0.append(lambda m=m: s1_v(1, m // 2, on_act=True))
            run_qb(0, f0)
            load_x(2, pieces=2)
            # qb=1: overlap with full s1(2) + s3(0)
            f1 = []
            for m in range(8):
                f1 += [lambda m=m: s1_qk_h(2, m, 0), lambda m=m: s1_qk_h(2, m, 1)]
                if m % 2 == 1:
                    f1.append(lambda m=m: s1_v(2, m // 2))
                    f1.append(lambda m=m: s3_ib(0, m // 2, 0))
                    f1.append(lambda m=m: s3_ib(0, m // 2, 1))
            run_qb(1, f1)
            load_x(3, pieces=2)
            # qb=2: overlap with s1(3) q-projection + s3(1)
            f2 = []
            for m in range(4):
                f2 += [lambda m=m: s1_qk_h(3, m, 0), lambda m=m: s1_qk_h(3, m, 1)]
                f2 += [lambda m=m: s3_ib(1, m, 0), lambda m=m: s3_ib(1, m, 1)]
            run_qb(2, f2)
            # qb=3: backfill with deferred s1(3) k-projection + v + s3(2)
            f3 = [lambda: s1_qk_h(3, 4, 0), lambda: s1_qk_h(3, 4, 1)]
            f3 += [lambda m=m: s1_v(3, m) for m in range(4)]
            for m in (1, 2, 3):
                f3 += [lambda m=m: s1_qk_h(3, 4 + m, 0),
                       lambda m=m: s1_qk_h(3, 4 + m, 1)]
            f3 += [lambda m=m: s3_ib(2, m // 2, m % 2) for m in range(4)]
            # front-load k(hp0) + all v tiles so hp0's diagonal PVs are fed;
            # hold two s3(2) units back to cover the last tail chain
            run_qb(3, f3, slots=[1, 2, 4, 5, 7, 8, 12, 16, 20, 26, 32, 38,
                                 44, 50, 54, 58],
                   late=[lambda m=m: s3_mt(2, m) for m in (2, 3)])
            for mt in range(4):
                s3_mt(3, mt, alt=True)

    nc.compile()
    return nc


# ---------------------------------------------------------------------------
# host side
# ---------------------------------------------------------------------------

_cache = {}


def _get_nc(S):
    if S not in _cache:
        _cache[S] = build_nc(S)
    return _cache[S]


def _shard_weights(w_qkv, w_out, g):
    """Per-head-group weight shards in device layouts."""
    w_qkv = np.asarray(w_qkv, dtype=np.float32)
    w_out = np.asarray(w_out, dtype=np.float32)
    r = slice(g * 512, (g + 1) * 512)
    wq = w_qkv[0:D][r]            # [512, 1024]
    wk = w_qkv[D:2 * D][r]
    wv = w_qkv[2 * D:3 * D][r]
    wqkT = np.ascontiguousarray(np.concatenate([wq, wk], axis=0).T).astype(BF16_NP)
    wvT = np.ascontiguousarray(wv.T).astype(BF16_NP)                  # [1024, 512]
    woutT = np.ascontiguousarray(w_out.T[r]).astype(BF16_NP)          # [512, 1024]
    return wqkT, wvT, woutT


def kernel(x, w_qkv, w_out):
    x = np.asarray(x, dtype=np.float32)
    B, S, _D = x.shape
    assert _D == D
    nc = _get_nc(S)

    shards = [_shard_weights(w_qkv, w_out, g) for g in range(2)]
    in_maps = []
    for core in range(8):
        b, g = core // 2, core % 2
        wqkT, wvT, woutT = shards[g]
        in_maps.append({
            "xT": np.ascontiguousarray(x[b].T).astype(BF16_NP),
            "wqkT": wqkT,
            "wvT": wvT,
            "woutT": woutT,
        })
    res = run_bass_kernel_spmd(nc, in_maps, list(range(8)))
    out = np.empty((B, S, D), dtype=np.float32)
    for b in range(B):
        out[b] = (res.results[2 * b]["outp"].astype(np.float32)
                  + res.results[2 * b + 1]["outp"].astype(np.float32))
    return out
